# revision 10
# baseline (speedup 1.0000x reference)
# Trainium2 Bass kernel for nn_BiMambaLayer (BiMamba block: AdaRMSNorm ->
# bidirectional Mamba -> out-proj residual -> AdaRMSNorm -> SwiGLU FFN).
#
# Sharding: 8 cores = 2 directions x 4 batches (core c: dir=c//4, b=c%4).
# Each core runs one direction of one sequence in its local time order
# (dir-1 cores receive time-flipped inputs, so the SPMD program is identical).
# Each core OUTPUTS the global-time half that equals its LOCAL second half,
# so the pair exchange of the mamba branch (the local first halves) overlaps
# with the second half's scan: after p7(0) each core contributes its local
# half0 (canonicalized to global time) to a pair AllReduce, keeps scanning,
# and only at the tail adds its own local half1 to the received partner half.
#
# Engine split for the scan phase (p5), per (e,n):
#   ACT:  dA = exp(A[e,n] * dt)           (exp table resident all phase)
#   Pool: dBu = dtu * B_n                 (gpsimd tensor_tensor)
#   DVE:  hs  = tensor_tensor_scan(dA, dBu)
#   DVE:  m_t = hs * C_n
#   PE:   y2_psum += I @ m_t              (accumulates over n per e-tile)
# Layout: feature-major everywhere [feature on partitions, time on free dim].
import numpy as np
import ml_dtypes

try:
    import ntff_hook_shim  # noqa: F401  (optional, enables trace in dev)
    ntff_hook_shim.install()
except Exception:
    pass

import concourse.bass as bass
import concourse.tile as tile
from concourse import bacc, mybir
from concourse.bass_utils import run_bass_kernel_spmd
from contextlib import ExitStack

F32 = mybir.dt.float32
BF16 = mybir.dt.bfloat16
AF = mybir.ActivationFunctionType
OP = mybir.AluOpType

D = 512          # d_model
ED = 1024        # d_inner
N = 16           # d_state
R = 32           # dt_rank
DC = 4           # d_conv
FF = 1536        # d_ff
B, L = 4, 1024
EPS = 1e-6
HL = L // 2      # tokens per half / per core in the FFN phase
NCORES = 8
ET = ED // 128   # 8 e-tiles
DT = D // 128    # 4 d-tiles
FT = FF // 128   # 12

_cache = {}


def _rev(ap):
    """Reverse the (last) free dim of a 2D AP."""
    a = list(ap.ap)
    assert len(a) == 2
    stride, n = a[1]
    return bass.AP(tensor=ap.tensor, offset=ap.offset + stride * (n - 1),
                  ap=[a[0], [-stride, n]])


def _bcast_mid(ap2d, k):
    """Insert a stride-0 middle free dim of size k into a 2D AP."""
    a = list(ap2d.ap)
    return bass.AP(tensor=ap2d.tensor, offset=ap2d.offset,
                  ap=[a[0], [0, k], a[1]])


def _bcast_row(src_row):
    """AP that reads a [1, F] SBUF row 128x (replication via stride-0 free dim)."""
    a = list(src_row.ap)
    return bass.AP(tensor=src_row.tensor, offset=src_row.offset,
                  ap=[a[0], [0, 128], a[1]])


def build(debug=False):
    nc = bacc.Bacc("TRN2", target_bir_lowering=False, debug=False,
                   num_devices=NCORES)

    def din(name, shape, dt=F32):
        return nc.dram_tensor(name, shape, dt, kind="ExternalInput").ap()

    ins = {}
    ins["xhT"] = din("xhT", [D, HL])               # x at MY global half (+ls1*blk_b)
    ins["xbT"] = din("xbT", [D, L], BF16)          # x[b].T local time (flipped dir1)
    ins["pnT"] = din("pnT", [2, L], BF16)          # phys_norm[b].T local time
    ins["pnoT"] = din("pnoT", [2, HL], BF16)       # phys_norm at MY half, global
    ins["dirmask"] = din("dirmask", [128, 2])      # col0=1-dir, col1=dir
    ins["eye"] = din("eye", [128, 128], BF16)
    for p in ("n1", "n2"):
        ins[p + "_w1T"] = din(p + "_w1T", [2, 2 * D], BF16)
        ins[p + "_b1"] = din(p + "_b1", [2 * D, 1])
        ins[p + "_w2T"] = din(p + "_w2T", [2 * D, 2 * D], BF16)
        ins[p + "_b2"] = din(p + "_b2", [2 * D, 1])
        ins[p + "_sc"] = din(p + "_sc", [D, 1])
    ins["in_wT"] = din("in_wT", [D, 2 * ED], BF16)
    ins["conv_w"] = din("conv_w", [ED, DC])
    ins["conv_b"] = din("conv_b", [ED, 1])
    ins["xp_wT"] = din("xp_wT", [ED, R + 2 * N], BF16)
    ins["dt_wT"] = din("dt_wT", [R, ED], BF16)
    ins["dt_b"] = din("dt_b", [ED, 1])
    ins["A"] = din("A", [ED, N])
    ins["Dsk"] = din("Dsk", [ED, 1])
    ins["WdT"] = din("WdT", [ED, D], BF16)         # (blk_half_dir @ out_w_dir).T
    ins["ls1"] = din("ls1", [D, 1])
    ins["fc1_wT"] = din("fc1_wT", [D, 2 * FF], BF16)
    ins["fc1_b"] = din("fc1_b", [2 * FF, 1])
    ins["fc2_wT"] = din("fc2_wT", [FF, D], BF16)
    ins["ls2"] = din("ls2", [D, 1])
    ins["c2T"] = din("c2T", [1, D], BF16)          # fc2_b as a row

    out_ap = nc.dram_tensor("out", [D, HL], F32, kind="ExternalOutput").ap()
    dbg = {}
    if debug:
        def dout(name, shape, dt=BF16):
            dbg[name] = nc.dram_tensor(name, shape, dt, kind="ExternalOutput").ap()
        dout("h_dbg", [D, L])
        dout("u_dbg", [ED, L])
        dout("sz_dbg", [ED, L])
        dout("dt_dbg", [ED, L])
        dout("bc_dbg", [2 * N, L])
        dout("y2_dbg", [ED, L])
        dout("x1_dbg", [D, HL], F32)

    with tile.TileContext(nc) as tc, ExitStack() as ctx:
        wpool = ctx.enter_context(tc.tile_pool(name="weights", bufs=1))
        wbig = ctx.enter_context(tc.tile_pool(name="wbig", bufs=2))
        arena = ctx.enter_context(tc.tile_pool(name="arena", bufs=1))
        rot = ctx.enter_context(tc.tile_pool(name="rot", bufs=2))
        psum = ctx.enter_context(tc.tile_pool(name="psum", bufs=2, space="PSUM"))
        dram = ctx.enter_context(tc.tile_pool(name="dram", bufs=1, space="DRAM"))

        _dma_rr = [nc.sync, nc.scalar]
        _rr = [0]

        def _wdma(out, in_):
            _dma_rr[_rr[0] % 2].dma_start(out, in_)
            _rr[0] += 1

        def w_big(name, K, M, src=None, q=None):
            t = wbig.tile([128, K // 128, M], BF16, name=name + "_sb", tag="wslab")
            if src is None:
                src = ins[name].rearrange("(a p) m -> p a m", p=128)
            if q is None:
                _wdma(t[:], src)
            else:
                q.dma_start(t[:], src)
            return t

        def w_perm(name, K, M, dt=BF16):
            t = wpool.tile([128, K // 128, M], dt, name=name + "_sb")
            _wdma(t[:], ins[name].rearrange("(a p) m -> p a m", p=128))
            return t

        def w_vec(name, K, dt=F32):
            t = wpool.tile([128, K // 128, 1], dt, name=name + "_sb")
            _wdma(t[:], ins[name].rearrange("(a p) o -> p a o", p=128))
            return t

        # ---- permanent small weights ----
        pn_sb = wpool.tile([2, L], BF16, name="pn_sb")
        nc.sync.dma_start(pn_sb[:], ins["pnT"][:])
        pno_sb = wpool.tile([2, HL], BF16, name="pno_sb")
        nc.sync.dma_start(pno_sb[:], ins["pnoT"][:])
        msk = wpool.tile([128, 2], F32, name="msk_sb")
        nc.sync.dma_start(msk[:], ins["dirmask"][:])
        eye_sb = wpool.tile([128, 128], BF16, name="eye_sb")
        nc.sync.dma_start(eye_sb[:], ins["eye"][:])
        n1_w1 = wpool.tile([2, 2 * D], BF16, name="n1_w1_sb")
        nc.sync.dma_start(n1_w1[:], ins["n1_w1T"][:])
        n2_w1 = wpool.tile([2, 2 * D], BF16, name="n2_w1_sb")
        nc.sync.dma_start(n2_w1[:], ins["n2_w1T"][:])
        n1_b1 = w_vec("n1_b1", 2 * D)
        n1_b2 = w_vec("n1_b2", 2 * D)
        n1_sc = w_vec("n1_sc", D)
        n2_b1 = w_vec("n2_b1", 2 * D)
        n2_b2 = w_vec("n2_b2", 2 * D)
        n2_sc = w_vec("n2_sc", D)
        conv_w = wpool.tile([128, ET, DC], F32, name="conv_w_sb")
        nc.sync.dma_start(conv_w[:], ins["conv_w"].rearrange("(a p) m -> p a m", p=128))
        conv_b = w_vec("conv_b", ED)
        xp_w = w_perm("xp_wT", ED, R + 2 * N)
        dt_w = wpool.tile([R, ED], BF16, name="dt_w_sb")
        nc.sync.dma_start(dt_w[:], ins["dt_wT"][:])
        dt_b = w_vec("dt_b", ED)
        A_sb = wpool.tile([128, ET, N], F32, name="A_sb")
        nc.sync.dma_start(A_sb[:], ins["A"].rearrange("(a p) m -> p a m", p=128))
        Dsk = w_vec("Dsk", ED)
        Wd = w_perm("WdT", ED, D)
        ls1 = w_vec("ls1", D)
        ls2 = w_vec("ls2", D)
        fc1_b = w_vec("fc1_b", 2 * FF)

        ones_col = wpool.tile([128, 1], BF16, name="ones_col")
        nc.vector.memset(ones_col[:], 1.0)
        ones_row = wpool.tile([1, HL], BF16, name="ones_row")
        nc.vector.memset(ones_row[:], 1.0)
        c2row = wpool.tile([1, D], BF16, name="c2row")
        nc.sync.dma_start(c2row[:], ins["c2T"][:])
        eps_t = wpool.tile([1, 1], F32, name="eps_t")
        nc.vector.memset(eps_t[:], EPS)

        # persistent / tag-shared big tiles
        p_bf = arena.tile([128, DT, L], BF16, name="p_bf", tag="p_bf")
        hlast = arena.tile([128, ET, N], F32, name="hlast", tag="hlast")
        x_half = arena.tile([128, DT, HL], F32, name="x_half", tag="s8x")
        nc.sync.dma_start(x_half[:],
                          ins["xhT"].rearrange("(a p) m -> p a m", p=128))
        dbc = arena.tile([R + 2 * N, L], BF16, name="dbc", tag="dbc")
        x1 = arena.tile([128, DT, HL], BF16, name="x1", tag="x1")
        u_dram = dram.tile([128, ET, L], BF16, name="u_dram")
        sz_dram = dram.tile([128, ET, L], BF16, name="sz_dram")
        cc_in = dram.tile([128, DT, L], BF16, name="cc_in")
        cc_out = dram.tile([128, DT, L], BF16, name="cc_out")

        n1_w2 = w_big("n1_w2T", 2 * D, 2 * D)
        in_w = w_big("in_wT", D, 2 * ED)

        # =============== P1: ada_norm 1 over full L (feature-major) =========
        xb = arena.tile([128, DT, L], BF16, name="xb", tag="s16c")
        nc.sync.dma_start(xb[:], ins["xbT"].rearrange("(a p) m -> p a m", p=128))
        h1 = arena.tile([128, DT, L], BF16, name="h1", tag="s8a")

        ms_ps = psum.tile([1, L], F32, name="ms_ps", tag="msps", bufs=1)
        for i in range(DT):
            sq = rot.tile([128, L], BF16, name="p1_sq", tag="ada_sq", bufs=2)
            nc.vector.tensor_tensor(sq[:], xb[:, i, :], xb[:, i, :], op=OP.mult)
            for f in range(L // 512):
                nc.tensor.matmul(ms_ps[:, f * 512:(f + 1) * 512],
                                 ones_col[:], sq[:, f * 512:(f + 1) * 512],
                                 start=(i == 0), stop=(i == DT - 1))
        lnm = rot.tile([1, L], BF16, name="p1_lnm", tag="ada_lnm", bufs=1)
        nc.scalar.activation(lnm[:], ms_ps[:], AF.Ln, bias=eps_t[:], scale=1.0 / D)
        rinv = rot.tile([1, L], BF16, name="p1_rinv", tag="ada_rinv", bufs=1)
        nc.scalar.activation(rinv[:], lnm[:], AF.Exp, bias=0.0, scale=-0.5)
        rb = rot.tile([128, L], BF16, name="p1_rb", tag="ada_rb", bufs=1)
        nc.sync.dma_start(rb[:], _bcast_row(rinv[0:1, :]))

        # cond MLP 1 (silu/tanh table window)
        sg = arena.tile([128, 2 * DT, L], BF16, name="p1_sg", tag="s16a")
        for m in range(2 * DT):
            ps = psum.tile([128, L], F32, name="p1_ps1", tag="ps")
            for f in range(L // 512):
                nc.tensor.matmul(ps[:, f * 512:(f + 1) * 512],
                                 n1_w1[:, m * 128:(m + 1) * 128],
                                 pn_sb[:, f * 512:f * 512 + 512],
                                 start=True, stop=True)
            nc.scalar.activation(sg[:, m, :], ps[:, :], AF.Silu,
                                 bias=n1_b1[:, m], scale=1.0)
        for i in range(DT):
            tgp = []
            for mm in (i, DT + i):
                tg = rot.tile([128, L], BF16, name="p1_tg", tag="ada_tg", bufs=2)
                ps = psum.tile([128, L], F32, name="p1_ps2", tag="ps")
                for f in range(L // 512):
                    for k in range(2 * DT):
                        nc.tensor.matmul(
                            ps[:, f * 512:(f + 1) * 512],
                            n1_w2[:, k, mm * 128:(mm + 1) * 128],
                            sg[:, k, f * 512:f * 512 + 512],
                            start=(k == 0), stop=(k == 2 * DT - 1))
                nc.scalar.activation(tg[:], ps[:, :], AF.Tanh,
                                     bias=n1_b2[:, mm], scale=1.0)
                tgp.append(tg)
            s1 = rot.tile([128, L], BF16, name="p1_s1", tag="ada_s1", bufs=1)
            nc.vector.tensor_scalar(s1[:], tgp[0][:], 0.5, n1_sc[:, i],
                                    op0=OP.mult, op1=OP.add)
            xr = rot.tile([128, L], BF16, name="p1_xr", tag="ada_xr", bufs=2)
            nc.vector.tensor_tensor(xr[:], xb[:, i, :], rb[:, :], op=OP.mult)
            hp = rot.tile([128, L], BF16, name="p1_hp", tag="ada_xr", bufs=2)
            nc.vector.tensor_tensor(hp[:], xr[:], s1[:], op=OP.mult)
            nc.vector.scalar_tensor_tensor(h1[:, i, :], tgp[1][:], 0.5, hp[:],
                                           op0=OP.mult, op1=OP.add)
        if debug:
            nc.sync.dma_start(dbg["h_dbg"].rearrange("(a p) m -> p a m", p=128),
                              h1[:])

        # =============== P2: in-proj, causal conv, silu -> u, sz ============
        for m in range(ET):
            ps = psum.tile([128, L], F32, name="p2ps", tag="ps")
            for f in range(L // 512):
                for k in range(DT):
                    nc.tensor.matmul(ps[:, f * 512:(f + 1) * 512],
                                     in_w[:, k, m * 128:(m + 1) * 128],
                                     h1[:, k, f * 512:f * 512 + 512],
                                     start=(k == 0), stop=(k == DT - 1))
            xs = rot.tile([128, DC - 1 + L], BF16, name="p2_xs", tag="xs", bufs=2)
            nc.vector.memset(xs[:, 0:DC - 1], 0.0)
            nc.scalar.copy(xs[:, DC - 1:], ps[:, :])
            acc = rot.tile([128, L], BF16, name="p2_acc", tag="u_t", bufs=2)
            nc.vector.tensor_scalar(acc[:], xs[:, 0:L], conv_w[:, m, 0:1],
                                    None, op0=OP.mult)
            for k in range(1, DC):
                acc2 = rot.tile([128, L], BF16, name="p2_acc2", tag="cva", bufs=2)
                nc.vector.scalar_tensor_tensor(acc2[:], xs[:, k:k + L],
                                               conv_w[:, m, k:k + 1], acc[:],
                                               op0=OP.mult, op1=OP.add)
                acc = acc2
            u_t = rot.tile([128, L], BF16, name="p2_u", tag="u_t", bufs=2)
            nc.scalar.activation(u_t[:], acc[:], AF.Silu,
                                 bias=conv_b[:, m], scale=1.0)
            nc.sync.dma_start(u_dram[:, m, :], u_t[:])
            ps2 = psum.tile([128, L], F32, name="p2ps2", tag="ps")
            for f in range(L // 512):
                for k in range(DT):
                    nc.tensor.matmul(ps2[:, f * 512:(f + 1) * 512],
                                     in_w[:, k, (ET + m) * 128:(ET + m + 1) * 128],
                                     h1[:, k, f * 512:f * 512 + 512],
                                     start=(k == 0), stop=(k == DT - 1))
            sz_t = rot.tile([128, L], BF16, name="p2_sz", tag="sz_t", bufs=2)
            nc.scalar.activation(sz_t[:], ps2[:, :], AF.Silu)
            nc.scalar.dma_start(sz_dram[:, m, :], sz_t[:])
        if debug:
            nc.gpsimd.dma_start(
                dbg["u_dbg"].rearrange("(a p) m -> p a m", p=128), u_dram[:])
            nc.gpsimd.dma_start(
                dbg["sz_dbg"].rearrange("(a p) m -> p a m", p=128), sz_dram[:])

        # =============== cond MLP 2 precompute (pn_out only; silu window) ===
        n2_w2 = w_big("n2_w2T", 2 * D, 2 * D)
        s2gb = arena.tile([128, 2 * DT, HL], BF16, name="s2gb", tag="s2gb")
        sg2 = arena.tile([128, 2 * DT, HL], BF16, name="sg2c", tag="s8b")
        for m in range(2 * DT):
            ps = psum.tile([128, HL], F32, name="c2ps1", tag="y2ps")
            nc.tensor.matmul(ps[:, :], n2_w1[:, m * 128:(m + 1) * 128],
                             pno_sb[:, :], start=True, stop=True)
            nc.scalar.activation(sg2[:, m, :], ps[:, :], AF.Silu,
                                 bias=n2_b1[:, m], scale=1.0)
        for i in range(DT):
            for j, mm in enumerate((i, DT + i)):
                ps = psum.tile([128, HL], F32, name="c2ps2", tag="y2ps")
                for k in range(2 * DT):
                    nc.tensor.matmul(ps[:, :], n2_w2[:, k, mm * 128:(mm + 1) * 128],
                                     sg2[:, k, :], start=(k == 0),
                                     stop=(k == 2 * DT - 1))
                tg = rot.tile([128, HL], BF16, name="c2_tg", tag="ada_tg", bufs=2)
                nc.scalar.activation(tg[:], ps[:, :], AF.Tanh,
                                     bias=n2_b2[:, mm], scale=1.0)
                if j == 0:
                    nc.vector.tensor_scalar(s2gb[:, i, :], tg[:], 0.5,
                                            n2_sc[:, i], op0=OP.mult, op1=OP.add)
                else:
                    nc.vector.tensor_scalar(s2gb[:, DT + i, :], tg[:], 0.5,
                                            None, op0=OP.mult)

        # =============== P3: x-proj -> dbc (full L, k-outer) ================
        ps3 = psum.tile([R + 2 * N, L], F32, name="p3ps", tag="ps")
        for k in range(ET):
            uk = rot.tile([128, L], BF16, name="p3_uk", tag="uk", bufs=2)
            nc.sync.dma_start(uk[:], u_dram[:, k, :])
            for f in range(L // 512):
                nc.tensor.matmul(ps3[:, f * 512:(f + 1) * 512],
                                 xp_w[:, k, :], uk[:, f * 512:f * 512 + 512],
                                 start=(k == 0), stop=(k == ET - 1))
        nc.scalar.copy(dbc[:], ps3[:, :])
        if debug:
            nc.sync.dma_start(dbg["bc_dbg"][:], dbc[R:R + 2 * N, :])

        # =============== P4: dt = softplus(...) ; dtu = dt*u (Pool) =========
        dt_bf = arena.tile([128, ET, L], BF16, name="dt_bf", tag="s16a")
        dtu = arena.tile([128, ET, L], BF16, name="dtu", tag="s16c")
        # softplus = ln(1 + exp(.)) batched in 2 groups of 4 e-tiles so the
        # exp/ln table loads amortize (exp and ln live in different sets).
        for g in range(2):
            exg = arena.tile([128, 4, L], BF16, name=f"p4_ex{g}", tag="s8a")
            for j in range(4):
                e = g * 4 + j
                ps = psum.tile([128, L], F32, name="p4ps", tag="ps")
                for f in range(L // 512):
                    nc.tensor.matmul(ps[:, f * 512:(f + 1) * 512],
                                     dt_w[:, e * 128:(e + 1) * 128],
                                     dbc[0:R, f * 512:f * 512 + 512],
                                     start=True, stop=True)
                nc.scalar.activation(exg[:, j, :], ps[:, :], AF.Exp,
                                     bias=dt_b[:, e], scale=1.0)
            for j in range(4):
                e = g * 4 + j
                nc.scalar.activation(dt_bf[:, e, :], exg[:, j, :], AF.Ln,
                                     bias=1.0, scale=1.0)
                uk = rot.tile([128, L], BF16, name="p4_uk", tag="uk", bufs=2)
                nc.sync.dma_start(uk[:], u_dram[:, e, :])
                nc.gpsimd.tensor_tensor(dtu[:, e, :], dt_bf[:, e, :], uk[:],
                                        op=OP.mult)
                if debug:
                    nc.sync.dma_start(
                        dbg["dt_dbg"].rearrange("(a p) m -> p a m", p=128)[:, e, :],
                        dt_bf[:, e, :])

        # =============== P5/P6/P7 per half ==================================
        # BCall layout: rows [0:N) = broadcast B_n, rows [N:2N) = broadcast C_n.
        # Broadcasts are built on PE (ones-row matmul) + ACT copy, keeping the
        # DMA queues free. dBu is batched 4 n's per Pool op to amortize the
        # Q7 launch + semaphore overhead. m_t lags the scan by one iteration
        # so it never reads an hs whose DVE pipe has not drained yet.
        def bcasts(hf, BCb, BCc):
            t0 = hf * HL
            tsl = slice(t0, t0 + HL)
            for n in range(N):
                nc.scalar.dma_start(BCb[:, n, :],
                                    _bcast_row(dbc[R + n:R + n + 1, tsl]))
            for n in range(N):
                nc.sync.dma_start(BCc[:, n, :],
                                  _bcast_row(dbc[R + N + n:R + N + n + 1, tsl]))

        def p567(hf, BCb, BCc, mid_dmas=()):
            t0 = hf * HL
            tsl = slice(t0, t0 + HL)
            y2sb = arena.tile([128, ET, HL], BF16, name=f"y2sb{hf}", tag="s8a")
            for e in range(ET):
                for de, dfn in mid_dmas:
                    if de == e:
                        dfn()
                y2ps = psum.tile([128, HL], F32, name="y2ps", tag="y2ps")
                hs_prev = None
                for n in range(N):
                    if n % 4 == 0:
                        dBu4 = rot.tile([128, 4, HL], BF16, name="dBu4",
                                        tag="cva", bufs=2)
                        nc.gpsimd.tensor_tensor(dBu4[:],
                                                _bcast_mid(dtu[:, e, tsl], 4),
                                                BCb[:, n:n + 4, :], op=OP.mult)
                    dA = rot.tile([128, HL], BF16, name="dA", tag="dA", bufs=2)
                    nc.scalar.activation(dA[:], dt_bf[:, e, tsl], AF.Exp,
                                         bias=0.0, scale=A_sb[:, e, n:n + 1])
                    hs = rot.tile([128, HL], BF16, name="hs", tag="hs", bufs=3)
                    if hf == 0:
                        nc.vector.tensor_tensor_scan(hs[:], dA[:],
                                                     dBu4[:, n % 4, :], 0.0,
                                                     op0=OP.mult, op1=OP.add)
                        nc.scalar.copy(hlast[:, e, n:n + 1], hs[:, HL - 1:HL])
                    else:
                        nc.vector.tensor_tensor_scan(hs[:], dA[:],
                                                     dBu4[:, n % 4, :],
                                                     hlast[:, e, n:n + 1],
                                                     op0=OP.mult, op1=OP.add)
                    if hs_prev is not None:
                        m_t = rot.tile([128, HL], BF16, name="m_t", tag="xs",
                                       bufs=2)
                        nc.vector.tensor_tensor(m_t[:], hs_prev[:],
                                                BCc[:, n - 1, :], op=OP.mult)
                        nc.tensor.matmul(y2ps[:, :], eye_sb[:, :], m_t[:],
                                         start=(n == 1), stop=False)
                    hs_prev = hs
                m_t = rot.tile([128, HL], BF16, name="m_t", tag="xs", bufs=2)
                nc.vector.tensor_tensor(m_t[:], hs_prev[:],
                                        BCc[:, N - 1, :], op=OP.mult)
                nc.tensor.matmul(y2ps[:, :], eye_sb[:, :], m_t[:],
                                 start=False, stop=True)
                uk = rot.tile([128, HL], BF16, name="p6_uk", tag="u_t", bufs=2)
                nc.sync.dma_start(uk[:], u_dram[:, e, tsl])
                szk = rot.tile([128, HL], BF16, name="p6_szk", tag="sz_t",
                               bufs=2)
                nc.scalar.dma_start(szk[:], sz_dram[:, e, tsl])
                ud = rot.tile([128, HL], BF16, name="p6_ud", tag="p6ud", bufs=2)
                nc.vector.scalar_tensor_tensor(ud[:], uk[:], Dsk[:, e],
                                               y2ps[:, :], op0=OP.mult,
                                               op1=OP.add)
                nc.vector.tensor_tensor(y2sb[:, e, :], ud[:], szk[:],
                                        op=OP.mult)
            if debug:
                nc.gpsimd.dma_start(
                    dbg["y2_dbg"].rearrange("(a p) m -> p a m", p=128)[:, :, tsl],
                    y2sb[:])
            for m in range(DT):
                ps = psum.tile([128, HL], F32, name="p7ps", tag="y2ps")
                for k in range(ET):
                    nc.tensor.matmul(ps[:, :], Wd[:, k, m * 128:(m + 1) * 128],
                                     y2sb[:, k, :], start=(k == 0),
                                     stop=(k == ET - 1))
                nc.scalar.copy(p_bf[:, m, tsl], ps[:, :])

        fc1a = wbig.tile([128, DT, FF], BF16, name="fc1a_sb", tag="wslab")
        fc1b = wbig.tile([128, DT, FF], BF16, name="fc1b_sb", tag="wslab")
        _fc1_src = ins["fc1_wT"].rearrange("(a p) m -> p a m", p=128)
        BC0b = arena.tile([128, N, HL], BF16, name="BC0b", tag="s8b")
        BC0c = arena.tile([128, N, HL], BF16, name="BC0c", tag="sCall")
        bcasts(0, BC0b, BC0c)
        p567(0, BC0b, BC0c, mid_dmas=(
            (4, lambda: nc.gpsimd.dma_start(fc1a[:], _fc1_src[:, :, 0:FF])),
            (6, lambda: nc.gpsimd.dma_start(fc1b[:], _fc1_src[:, :, FF:2 * FF])),
        ))

        # ====== early pair exchange: contribute local half0 in global time ==
        for m in range(DT):
            cc0 = rot.tile([128, HL], BF16, name="cc0", tag="ada_sq", bufs=2)
            nc.vector.tensor_scalar(cc0[:], p_bf[:, m, 0:HL], msk[:, 0:1],
                                    None, op0=OP.mult)
            nc.sync.dma_start(cc_in[:, m, 0:HL], cc0[:])
            cc1 = rot.tile([128, HL], BF16, name="cc1", tag="ada_rb", bufs=1)
            nc.vector.tensor_scalar(cc1[:], _rev(p_bf[:, m, 0:HL]), msk[:, 1:2],
                                    None, op0=OP.mult)
            nc.sync.dma_start(cc_in[:, m, HL:L], cc1[:])
        nc.gpsimd.collective_compute(
            "AllReduce", OP.add,
            replica_groups=[[0, 4], [1, 5], [2, 6], [3, 7]],
            ins=[cc_in.opt()], outs=[cc_out.opt()])

        BC1b = arena.tile([128, N, HL], BF16, name="BC1b", tag="s8b")
        BC1c = arena.tile([128, N, HL], BF16, name="BC1c", tag="sCall")
        bcasts(1, BC1b, BC1c)
        p567(1, BC1b, BC1c)
        fc2_w = w_big("fc2_wT", FF, D, q=nc.gpsimd)

        # =============== P9: combine halves + residual ======================
        pg = arena.tile([128, DT, L], BF16, name="pg", tag="s8b")
        nc.sync.dma_start(pg[:], cc_out[:])
        for m in range(DT):
            t0 = rot.tile([128, HL], BF16, name="sel_t0", tag="ada_xr", bufs=2)
            nc.vector.tensor_scalar(t0[:], p_bf[:, m, HL:L], msk[:, 0:1],
                                    None, op0=OP.mult)
            t1 = rot.tile([128, HL], BF16, name="sel_t1", tag="ada_xr", bufs=2)
            nc.vector.scalar_tensor_tensor(t1[:], _rev(p_bf[:, m, HL:L]),
                                           msk[:, 1:2], t0[:],
                                           op0=OP.mult, op1=OP.add)
            t2 = rot.tile([128, HL], BF16, name="sel_t2", tag="ada_xr", bufs=2)
            nc.vector.scalar_tensor_tensor(t2[:], pg[:, m, HL:L], msk[:, 0:1],
                                           t1[:], op0=OP.mult, op1=OP.add)
            t3 = rot.tile([128, HL], BF16, name="sel_t3", tag="ada_xr", bufs=2)
            nc.vector.scalar_tensor_tensor(t3[:], pg[:, m, 0:HL], msk[:, 1:2],
                                           t2[:], op0=OP.mult, op1=OP.add)
            nc.vector.scalar_tensor_tensor(x1[:, m, :], t3[:], ls1[:, m],
                                           x_half[:, m, :], op0=OP.mult,
                                           op1=OP.add)
        if debug:
            x1d = rot.tile([128, DT, HL], F32, name="x1d", tag="x1d", bufs=1)
            for m in range(DT):
                nc.vector.tensor_copy(x1d[:, m, :], x1[:, m, :])
            nc.sync.dma_start(
                dbg["x1_dbg"].rearrange("(a p) m -> p a m", p=128), x1d[:])

        # =============== P10: ada_norm 2 (precomputed gamma/beta) ===========
        ms2 = psum.tile([1, HL], F32, name="ms2", tag="msps", bufs=1)
        for i in range(DT):
            sq = rot.tile([128, HL], BF16, name="p10_sq", tag="ada_sq", bufs=2)
            nc.vector.tensor_tensor(sq[:], x1[:, i, :], x1[:, i, :], op=OP.mult)
            nc.tensor.matmul(ms2[:, :], ones_col[:], sq[:],
                             start=(i == 0), stop=(i == DT - 1))
        lnm2 = rot.tile([1, HL], BF16, name="lnm2", tag="ada_lnm", bufs=1)
        nc.scalar.activation(lnm2[:], ms2[:], AF.Ln, bias=eps_t[:], scale=1.0 / D)
        rinv2 = rot.tile([1, HL], BF16, name="rinv2", tag="ada_rinv", bufs=1)
        nc.scalar.activation(rinv2[:], lnm2[:], AF.Exp, bias=0.0, scale=-0.5)
        rb2 = rot.tile([128, HL], BF16, name="rb2", tag="ada_rb", bufs=1)
        nc.sync.dma_start(rb2[:], _bcast_row(rinv2[0:1, :]))
        h2 = arena.tile([128, DT, HL], BF16, name="h2", tag="sCall")
        for i in range(DT):
            xr = rot.tile([128, HL], BF16, name="p10_xr", tag="ada_xr", bufs=2)
            nc.vector.tensor_tensor(xr[:], x1[:, i, :], rb2[:, :], op=OP.mult)
            hp = rot.tile([128, HL], BF16, name="p10_hp", tag="ada_xr", bufs=2)
            nc.vector.tensor_tensor(hp[:], xr[:], s2gb[:, i, :], op=OP.mult)
            nc.vector.tensor_tensor(h2[:, i, :], hp[:], s2gb[:, DT + i, :],
                                    op=OP.add)

        # =============== P11: SwiGLU FFN ====================================
        sgf = arena.tile([128, FT, HL], BF16, name="sgf", tag="s16a")
        gv = arena.tile([128, FT, HL], BF16, name="gv", tag="s16c")
        for m in range(2 * FT):
            ps = psum.tile([128, HL], F32, name="p11ps", tag="y2ps")
            fw = fc1a if m < FT else fc1b
            mc = m if m < FT else m - FT
            for k in range(DT):
                nc.tensor.matmul(ps[:, :], fw[:, k, mc * 128:(mc + 1) * 128],
                                 h2[:, k, :], start=(k == 0), stop=(k == DT - 1))
            if m < FT:
                nc.scalar.activation(sgf[:, m, :], ps[:, :], AF.Silu,
                                     bias=fc1_b[:, m], scale=1.0)
            else:
                vv = rot.tile([128, HL], BF16, name="p11_vv", tag="ada_tg", bufs=2)
                nc.scalar.activation(vv[:], ps[:, :], AF.Identity,
                                     bias=fc1_b[:, m], scale=1.0)
                nc.vector.tensor_tensor(gv[:, m - FT, :], sgf[:, m - FT, :],
                                        vv[:], op=OP.mult)
        out_sb = arena.tile([128, DT, HL], F32, name="out_sb", tag="s8x")
        for m in range(DT):
            ps = psum.tile([128, HL], F32, name="p12ps", tag="y2ps")
            for k in range(FT):
                nc.tensor.matmul(ps[:, :], fc2_w[:, k, m * 128:(m + 1) * 128],
                                 gv[:, k, :], start=(k == 0), stop=False)
            nc.tensor.matmul(ps[:, :], c2row[:, m * 128:(m + 1) * 128],
                             ones_row[:], start=False, stop=True)
            nc.vector.scalar_tensor_tensor(out_sb[:, m, :], ps[:, :], ls2[:, m],
                                           x1[:, m, :], op0=OP.mult, op1=OP.add)
        nc.sync.dma_start(out_ap.rearrange("(a p) m -> p a m", p=128), out_sb[:])

    nc.compile()
    return nc, dbg


def _prep_inputs(inputs):
    """Host-side: per-core input dicts."""
    f32 = np.float32
    bf = ml_dtypes.bfloat16
    x = np.asarray(inputs["x"], f32)
    pn = np.asarray(inputs["phys_norm"], f32)
    blk_w = np.asarray(inputs["blk_w"], f32)
    ls1v = np.asarray(inputs["ls1"], f32)
    blk_b = np.asarray(inputs["blk_b"], f32)
    in_maps = []
    for c in range(NCORES):
        d, b = c // 4, c % 4
        xb = x[b] if d == 0 else x[b, ::-1]
        pnb = pn[b] if d == 0 else pn[b, ::-1]
        osl = slice(HL, L) if d == 0 else slice(0, HL)  # my output half (global)
        xh = x[b, osl] + (ls1v * blk_b)[None, :]
        Wd = blk_w[:, d * D:(d + 1) * D] @ np.asarray(inputs["m_out_w"][d], f32)
        m = {
            "xhT": np.ascontiguousarray(xh.T),
            "xbT": np.ascontiguousarray(xb.T).astype(bf),
            "pnT": np.ascontiguousarray(pnb.T).astype(bf),
            "pnoT": np.ascontiguousarray(pn[b, osl].T).astype(bf),
            "dirmask": np.tile(np.array([[1.0 - d, float(d)]], f32), (128, 1)),
            "eye": np.eye(128, dtype=f32).astype(bf),
            "in_wT": np.ascontiguousarray(inputs["m_in_w"][d].T).astype(bf),
            "conv_w": np.asarray(inputs["m_conv_w"][d], f32),
            "conv_b": np.asarray(inputs["m_conv_b"][d], f32).reshape(ED, 1),
            "xp_wT": np.ascontiguousarray(inputs["m_xproj_w"][d].T).astype(bf),
            "dt_wT": np.ascontiguousarray(inputs["m_dt_w"][d].T).astype(bf),
            "dt_b": np.asarray(inputs["m_dt_b"][d], f32).reshape(ED, 1),
            "A": (-np.exp(np.asarray(inputs["m_A_log"][d], f32))),
            "Dsk": np.asarray(inputs["m_D"][d], f32).reshape(ED, 1),
            "WdT": np.ascontiguousarray(Wd.T).astype(bf),
            "ls1": ls1v.reshape(D, 1),
            "fc1_wT": np.ascontiguousarray(inputs["fc1_w"].T).astype(bf),
            "fc1_b": np.asarray(inputs["fc1_b"], f32).reshape(2 * FF, 1),
            "fc2_wT": np.ascontiguousarray(inputs["fc2_w"].T).astype(bf),
            "ls2": np.asarray(inputs["ls2"], f32).reshape(D, 1),
            "c2T": np.asarray(inputs["fc2_b"], f32).reshape(1, D).astype(bf),
        }
        for p in ("n1", "n2"):
            m[p + "_w1T"] = np.ascontiguousarray(inputs[p + "_w1"].T).astype(bf)
            m[p + "_b1"] = np.asarray(inputs[p + "_b1"], f32).reshape(2 * D, 1)
            m[p + "_w2T"] = np.ascontiguousarray(inputs[p + "_w2"].T).astype(bf)
            m[p + "_b2"] = np.asarray(inputs[p + "_b2"], f32).reshape(2 * D, 1)
            m[p + "_sc"] = np.asarray(inputs[p + "_scale"], f32).reshape(D, 1)
        in_maps.append(m)
    return in_maps


def run(inputs, debug=False, trace=False):
    key = ("dbg" if debug else "lean")
    if key not in _cache:
        _cache[key] = build(debug=debug)
    nc, dbg = _cache[key]
    in_maps = _prep_inputs(inputs)
    res = run_bass_kernel_spmd(nc, in_maps, core_ids=list(range(NCORES)),
                               trace=trace)
    out = np.zeros((B, L, D), np.float32)
    for c in range(NCORES):
        d, b = c // 4, c % 4
        o = res.results[c]["out"]  # [D, HL], global order, my half
        if d == 0:
            out[b, HL:L] = o.T
        else:
            out[b, 0:HL] = o.T
    return out, res


def kernel(**inputs):
    out, _ = run(inputs, debug=False, trace=False)
    return out


# revision 11
# speedup vs baseline: 1.0006x; 1.0006x over previous
# Trainium2 Bass kernel for nn_BiMambaLayer (BiMamba block: AdaRMSNorm ->
# bidirectional Mamba -> out-proj residual -> AdaRMSNorm -> SwiGLU FFN).
#
# Sharding: 8 cores = 2 directions x 4 batches (core c: dir=c//4, b=c%4).
# Each core runs one direction of one sequence in its local time order
# (dir-1 cores receive time-flipped inputs, so the SPMD program is identical).
# Each core OUTPUTS the global-time half that equals its LOCAL second half,
# so the pair exchange of the mamba branch (the local first halves) overlaps
# with the second half's scan: after p7(0) each core contributes its local
# half0 (canonicalized to global time) to a pair AllReduce, keeps scanning,
# and only at the tail adds its own local half1 to the received partner half.
#
# Engine split for the scan phase (p5), per (e,n):
#   ACT:  dA = exp(A[e,n] * dt)           (exp table resident all phase)
#   Pool: dBu = dtu * B_n                 (gpsimd tensor_tensor)
#   DVE:  hs  = tensor_tensor_scan(dA, dBu)
#   DVE:  m_t = hs * C_n
#   PE:   y2_psum += I @ m_t              (accumulates over n per e-tile)
# Layout: feature-major everywhere [feature on partitions, time on free dim].
import numpy as np
import ml_dtypes

try:
    import ntff_hook_shim  # noqa: F401  (optional, enables trace in dev)
    ntff_hook_shim.install()
except Exception:
    pass

import concourse.bass as bass
import concourse.tile as tile
from concourse import bacc, mybir
from concourse.bass_utils import run_bass_kernel_spmd
from contextlib import ExitStack

F32 = mybir.dt.float32
BF16 = mybir.dt.bfloat16
AF = mybir.ActivationFunctionType
OP = mybir.AluOpType

D = 512          # d_model
ED = 1024        # d_inner
N = 16           # d_state
R = 32           # dt_rank
DC = 4           # d_conv
FF = 1536        # d_ff
B, L = 4, 1024
EPS = 1e-6
HL = L // 2      # tokens per half / per core in the FFN phase
NCORES = 8
ET = ED // 128   # 8 e-tiles
DT = D // 128    # 4 d-tiles
FT = FF // 128   # 12

_cache = {}


def _rev(ap):
    """Reverse the (last) free dim of a 2D AP."""
    a = list(ap.ap)
    assert len(a) == 2
    stride, n = a[1]
    return bass.AP(tensor=ap.tensor, offset=ap.offset + stride * (n - 1),
                  ap=[a[0], [-stride, n]])


def _bcast_mid(ap2d, k):
    """Insert a stride-0 middle free dim of size k into a 2D AP."""
    a = list(ap2d.ap)
    return bass.AP(tensor=ap2d.tensor, offset=ap2d.offset,
                  ap=[a[0], [0, k], a[1]])


def _bcast_row(src_row):
    """AP that reads a [1, F] SBUF row 128x (replication via stride-0 free dim)."""
    a = list(src_row.ap)
    return bass.AP(tensor=src_row.tensor, offset=src_row.offset,
                  ap=[a[0], [0, 128], a[1]])


def build(debug=False):
    nc = bacc.Bacc("TRN2", target_bir_lowering=False, debug=False,
                   num_devices=NCORES)

    def din(name, shape, dt=F32):
        return nc.dram_tensor(name, shape, dt, kind="ExternalInput").ap()

    ins = {}
    ins["xhT"] = din("xhT", [D, HL])               # x at MY global half (+ls1*blk_b)
    ins["xbT"] = din("xbT", [D, L], BF16)          # x[b].T local time (flipped dir1)
    ins["pnT"] = din("pnT", [2, L], BF16)          # phys_norm[b].T local time
    ins["pnoT"] = din("pnoT", [2, HL], BF16)       # phys_norm at MY half, global
    ins["dirmask"] = din("dirmask", [128, 2])      # col0=1-dir, col1=dir
    ins["eye"] = din("eye", [128, 128], BF16)
    for p in ("n1", "n2"):
        ins[p + "_w1T"] = din(p + "_w1T", [2, 2 * D], BF16)
        ins[p + "_b1"] = din(p + "_b1", [2 * D, 1])
        ins[p + "_w2T"] = din(p + "_w2T", [2 * D, 2 * D], BF16)
        ins[p + "_b2"] = din(p + "_b2", [2 * D, 1])
        ins[p + "_sc"] = din(p + "_sc", [D, 1])
    ins["in_wT"] = din("in_wT", [D, 2 * ED], BF16)
    ins["conv_w"] = din("conv_w", [ED, DC])
    ins["conv_b"] = din("conv_b", [ED, 1])
    ins["xp_wT"] = din("xp_wT", [ED, R + 2 * N], BF16)
    ins["dt_wT"] = din("dt_wT", [R, ED], BF16)
    ins["dt_b"] = din("dt_b", [ED, 1])
    ins["A"] = din("A", [ED, N])
    ins["Dsk"] = din("Dsk", [ED, 1])
    ins["WdT"] = din("WdT", [ED, D], BF16)         # (blk_half_dir @ out_w_dir).T
    ins["ls1"] = din("ls1", [D, 1])
    ins["fc1_wT"] = din("fc1_wT", [D, 2 * FF], BF16)
    ins["fc1_b"] = din("fc1_b", [2 * FF, 1])
    ins["fc2_wT"] = din("fc2_wT", [FF, D], BF16)
    ins["ls2"] = din("ls2", [D, 1])
    ins["c2T"] = din("c2T", [1, D], BF16)          # fc2_b as a row

    out_ap = nc.dram_tensor("out", [D, HL], F32, kind="ExternalOutput").ap()
    dbg = {}
    if debug:
        def dout(name, shape, dt=BF16):
            dbg[name] = nc.dram_tensor(name, shape, dt, kind="ExternalOutput").ap()
        dout("h_dbg", [D, L])
        dout("u_dbg", [ED, L])
        dout("sz_dbg", [ED, L])
        dout("dt_dbg", [ED, L])
        dout("bc_dbg", [2 * N, L])
        dout("y2_dbg", [ED, L])
        dout("x1_dbg", [D, HL], F32)

    with tile.TileContext(nc) as tc, ExitStack() as ctx:
        wpool = ctx.enter_context(tc.tile_pool(name="weights", bufs=1))
        wbig = ctx.enter_context(tc.tile_pool(name="wbig", bufs=2))
        arena = ctx.enter_context(tc.tile_pool(name="arena", bufs=1))
        rot = ctx.enter_context(tc.tile_pool(name="rot", bufs=2))
        psum = ctx.enter_context(tc.tile_pool(name="psum", bufs=2, space="PSUM"))
        dram = ctx.enter_context(tc.tile_pool(name="dram", bufs=1, space="DRAM"))

        _dma_rr = [nc.sync, nc.scalar]
        _rr = [0]

        def _wdma(out, in_):
            _dma_rr[_rr[0] % 2].dma_start(out, in_)
            _rr[0] += 1

        def w_big(name, K, M, src=None, q=None):
            t = wbig.tile([128, K // 128, M], BF16, name=name + "_sb", tag="wslab")
            if src is None:
                src = ins[name].rearrange("(a p) m -> p a m", p=128)
            if q is None:
                _wdma(t[:], src)
            else:
                q.dma_start(t[:], src)
            return t

        def w_perm(name, K, M, dt=BF16):
            t = wpool.tile([128, K // 128, M], dt, name=name + "_sb")
            _wdma(t[:], ins[name].rearrange("(a p) m -> p a m", p=128))
            return t

        def w_vec(name, K, dt=F32):
            t = wpool.tile([128, K // 128, 1], dt, name=name + "_sb")
            _wdma(t[:], ins[name].rearrange("(a p) o -> p a o", p=128))
            return t

        # ---- permanent small weights ----
        pn_sb = wpool.tile([2, L], BF16, name="pn_sb")
        nc.sync.dma_start(pn_sb[:], ins["pnT"][:])
        pno_sb = wpool.tile([2, HL], BF16, name="pno_sb")
        nc.sync.dma_start(pno_sb[:], ins["pnoT"][:])
        msk = wpool.tile([128, 2], F32, name="msk_sb")
        nc.sync.dma_start(msk[:], ins["dirmask"][:])
        eye_sb = wpool.tile([128, 128], BF16, name="eye_sb")
        nc.sync.dma_start(eye_sb[:], ins["eye"][:])
        n1_w1 = wpool.tile([2, 2 * D], BF16, name="n1_w1_sb")
        nc.sync.dma_start(n1_w1[:], ins["n1_w1T"][:])
        n2_w1 = wpool.tile([2, 2 * D], BF16, name="n2_w1_sb")
        nc.sync.dma_start(n2_w1[:], ins["n2_w1T"][:])
        n1_b1 = w_vec("n1_b1", 2 * D)
        n1_b2 = w_vec("n1_b2", 2 * D)
        n1_sc = w_vec("n1_sc", D)
        n2_b1 = w_vec("n2_b1", 2 * D)
        n2_b2 = w_vec("n2_b2", 2 * D)
        n2_sc = w_vec("n2_sc", D)
        conv_w = wpool.tile([128, ET, DC], F32, name="conv_w_sb")
        nc.sync.dma_start(conv_w[:], ins["conv_w"].rearrange("(a p) m -> p a m", p=128))
        conv_b = w_vec("conv_b", ED)
        xp_w = w_perm("xp_wT", ED, R + 2 * N)
        dt_w = wpool.tile([R, ED], BF16, name="dt_w_sb")
        nc.sync.dma_start(dt_w[:], ins["dt_wT"][:])
        dt_b = w_vec("dt_b", ED)
        A_sb = wpool.tile([128, ET, N], F32, name="A_sb")
        nc.sync.dma_start(A_sb[:], ins["A"].rearrange("(a p) m -> p a m", p=128))
        Dsk = w_vec("Dsk", ED)
        Wd = w_perm("WdT", ED, D)
        ls1 = w_vec("ls1", D)
        ls2 = w_vec("ls2", D)
        fc1_b = w_vec("fc1_b", 2 * FF)

        ones_col = wpool.tile([128, 1], BF16, name="ones_col")
        nc.vector.memset(ones_col[:], 1.0)
        ones_row = wpool.tile([1, HL], BF16, name="ones_row")
        nc.vector.memset(ones_row[:], 1.0)
        c2row = wpool.tile([1, D], BF16, name="c2row")
        nc.sync.dma_start(c2row[:], ins["c2T"][:])
        eps_t = wpool.tile([1, 1], F32, name="eps_t")
        nc.vector.memset(eps_t[:], EPS)

        # persistent / tag-shared big tiles
        p_bf = arena.tile([128, DT, L], BF16, name="p_bf", tag="p_bf")
        hlast = arena.tile([128, ET, N], F32, name="hlast", tag="hlast")
        x_half = arena.tile([128, DT, HL], F32, name="x_half", tag="s8x")
        nc.sync.dma_start(x_half[:],
                          ins["xhT"].rearrange("(a p) m -> p a m", p=128))
        dbc = arena.tile([R + 2 * N, L], BF16, name="dbc", tag="dbc")
        x1 = arena.tile([128, DT, HL], BF16, name="x1", tag="x1")
        u_dram = dram.tile([128, ET, L], BF16, name="u_dram")
        sz_dram = dram.tile([128, ET, L], BF16, name="sz_dram")
        cc_in = dram.tile([128, DT, L], BF16, name="cc_in")
        cc_out = dram.tile([128, DT, L], BF16, name="cc_out")

        n1_w2 = w_big("n1_w2T", 2 * D, 2 * D)
        in_w = w_big("in_wT", D, 2 * ED)

        # =============== P1: ada_norm 1 over full L (feature-major) =========
        xb = arena.tile([128, DT, L], BF16, name="xb", tag="s16c")
        nc.sync.dma_start(xb[:], ins["xbT"].rearrange("(a p) m -> p a m", p=128))
        h1 = arena.tile([128, DT, L], BF16, name="h1", tag="s8a")

        ms_ps = psum.tile([1, L], F32, name="ms_ps", tag="msps", bufs=1)
        for i in range(DT):
            sq = rot.tile([128, L], BF16, name="p1_sq", tag="ada_sq", bufs=2)
            nc.vector.tensor_tensor(sq[:], xb[:, i, :], xb[:, i, :], op=OP.mult)
            for f in range(L // 512):
                nc.tensor.matmul(ms_ps[:, f * 512:(f + 1) * 512],
                                 ones_col[:], sq[:, f * 512:(f + 1) * 512],
                                 start=(i == 0), stop=(i == DT - 1))
        lnm = rot.tile([1, L], BF16, name="p1_lnm", tag="ada_lnm", bufs=1)
        nc.scalar.activation(lnm[:], ms_ps[:], AF.Ln, bias=eps_t[:], scale=1.0 / D)
        rinv = rot.tile([1, L], BF16, name="p1_rinv", tag="ada_rinv", bufs=1)
        nc.scalar.activation(rinv[:], lnm[:], AF.Exp, bias=0.0, scale=-0.5)
        rb = rot.tile([128, L], BF16, name="p1_rb", tag="ada_rb", bufs=1)
        nc.sync.dma_start(rb[:], _bcast_row(rinv[0:1, :]))

        # cond MLP 1 (silu/tanh table window)
        sg = arena.tile([128, 2 * DT, L], BF16, name="p1_sg", tag="s16a")
        for m in range(2 * DT):
            ps = psum.tile([128, L], F32, name="p1_ps1", tag="ps")
            for f in range(L // 512):
                nc.tensor.matmul(ps[:, f * 512:(f + 1) * 512],
                                 n1_w1[:, m * 128:(m + 1) * 128],
                                 pn_sb[:, f * 512:f * 512 + 512],
                                 start=True, stop=True)
            nc.scalar.activation(sg[:, m, :], ps[:, :], AF.Silu,
                                 bias=n1_b1[:, m], scale=1.0)
        for i in range(DT):
            tgp = []
            for mm in (i, DT + i):
                tg = rot.tile([128, L], BF16, name="p1_tg", tag="ada_tg", bufs=2)
                ps = psum.tile([128, L], F32, name="p1_ps2", tag="ps")
                for f in range(L // 512):
                    for k in range(2 * DT):
                        nc.tensor.matmul(
                            ps[:, f * 512:(f + 1) * 512],
                            n1_w2[:, k, mm * 128:(mm + 1) * 128],
                            sg[:, k, f * 512:f * 512 + 512],
                            start=(k == 0), stop=(k == 2 * DT - 1))
                nc.scalar.activation(tg[:], ps[:, :], AF.Tanh,
                                     bias=n1_b2[:, mm], scale=1.0)
                tgp.append(tg)
            s1 = rot.tile([128, L], BF16, name="p1_s1", tag="sz_t", bufs=2)
            nc.vector.tensor_scalar(s1[:], tgp[0][:], 0.5, n1_sc[:, i],
                                    op0=OP.mult, op1=OP.add)
            xr = rot.tile([128, L], BF16, name="p1_xr", tag="ada_xr", bufs=2)
            nc.vector.tensor_tensor(xr[:], xb[:, i, :], rb[:, :], op=OP.mult)
            hp = rot.tile([128, L], BF16, name="p1_hp", tag="ada_xr", bufs=2)
            nc.vector.tensor_tensor(hp[:], xr[:], s1[:], op=OP.mult)
            nc.vector.scalar_tensor_tensor(h1[:, i, :], tgp[1][:], 0.5, hp[:],
                                           op0=OP.mult, op1=OP.add)
        if debug:
            nc.sync.dma_start(dbg["h_dbg"].rearrange("(a p) m -> p a m", p=128),
                              h1[:])

        # =============== P2: in-proj, causal conv, silu -> u, sz ============
        # x-proj (old p3) folded in: ps3 accumulates xp_w[m] @ u[m] as each
        # u-tile is produced, so dbc is ready right after the m-loop.
        ps3 = psum.tile([R + 2 * N, L], F32, name="p3ps", tag="msps", bufs=1)
        for m in range(ET):
            ps = psum.tile([128, L], F32, name="p2ps", tag="ps")
            for f in range(L // 512):
                for k in range(DT):
                    nc.tensor.matmul(ps[:, f * 512:(f + 1) * 512],
                                     in_w[:, k, m * 128:(m + 1) * 128],
                                     h1[:, k, f * 512:f * 512 + 512],
                                     start=(k == 0), stop=(k == DT - 1))
            xs = rot.tile([128, DC - 1 + L], BF16, name="p2_xs", tag="xs", bufs=2)
            nc.vector.memset(xs[:, 0:DC - 1], 0.0)
            nc.scalar.copy(xs[:, DC - 1:], ps[:, :])
            acc = rot.tile([128, L], BF16, name="p2_acc", tag="u_t", bufs=2)
            nc.vector.tensor_scalar(acc[:], xs[:, 0:L], conv_w[:, m, 0:1],
                                    None, op0=OP.mult)
            for k in range(1, DC):
                acc2 = rot.tile([128, L], BF16, name="p2_acc2", tag="cva", bufs=3)
                nc.vector.scalar_tensor_tensor(acc2[:], xs[:, k:k + L],
                                               conv_w[:, m, k:k + 1], acc[:],
                                               op0=OP.mult, op1=OP.add)
                acc = acc2
            u_t = rot.tile([128, L], BF16, name="p2_u", tag="u_t", bufs=2)
            nc.scalar.activation(u_t[:], acc[:], AF.Silu,
                                 bias=conv_b[:, m], scale=1.0)
            nc.sync.dma_start(u_dram[:, m, :], u_t[:])
            for f in range(L // 512):
                nc.tensor.matmul(ps3[:, f * 512:(f + 1) * 512],
                                 xp_w[:, m, :], u_t[:, f * 512:f * 512 + 512],
                                 start=(m == 0), stop=(m == ET - 1))
            ps2 = psum.tile([128, L], F32, name="p2ps2", tag="ps")
            for f in range(L // 512):
                for k in range(DT):
                    nc.tensor.matmul(ps2[:, f * 512:(f + 1) * 512],
                                     in_w[:, k, (ET + m) * 128:(ET + m + 1) * 128],
                                     h1[:, k, f * 512:f * 512 + 512],
                                     start=(k == 0), stop=(k == DT - 1))
            sz_t = rot.tile([128, L], BF16, name="p2_sz", tag="sz_t", bufs=2)
            nc.scalar.activation(sz_t[:], ps2[:, :], AF.Silu)
            nc.scalar.dma_start(sz_dram[:, m, :], sz_t[:])
        if debug:
            nc.gpsimd.dma_start(
                dbg["u_dbg"].rearrange("(a p) m -> p a m", p=128), u_dram[:])
            nc.gpsimd.dma_start(
                dbg["sz_dbg"].rearrange("(a p) m -> p a m", p=128), sz_dram[:])

        # =============== cond MLP 2 precompute (pn_out only; silu window) ===
        n2_w2 = w_big("n2_w2T", 2 * D, 2 * D)
        s2gb = arena.tile([128, 2 * DT, HL], BF16, name="s2gb", tag="s2gb")
        sg2 = arena.tile([128, 2 * DT, HL], BF16, name="sg2c", tag="s8b")
        for m in range(2 * DT):
            ps = psum.tile([128, HL], F32, name="c2ps1", tag="y2ps")
            nc.tensor.matmul(ps[:, :], n2_w1[:, m * 128:(m + 1) * 128],
                             pno_sb[:, :], start=True, stop=True)
            nc.scalar.activation(sg2[:, m, :], ps[:, :], AF.Silu,
                                 bias=n2_b1[:, m], scale=1.0)
        for i in range(DT):
            for j, mm in enumerate((i, DT + i)):
                ps = psum.tile([128, HL], F32, name="c2ps2", tag="y2ps")
                for k in range(2 * DT):
                    nc.tensor.matmul(ps[:, :], n2_w2[:, k, mm * 128:(mm + 1) * 128],
                                     sg2[:, k, :], start=(k == 0),
                                     stop=(k == 2 * DT - 1))
                tg = rot.tile([128, HL], BF16, name="c2_tg", tag="ada_tg", bufs=2)
                nc.scalar.activation(tg[:], ps[:, :], AF.Tanh,
                                     bias=n2_b2[:, mm], scale=1.0)
                if j == 0:
                    nc.vector.tensor_scalar(s2gb[:, i, :], tg[:], 0.5,
                                            n2_sc[:, i], op0=OP.mult, op1=OP.add)
                else:
                    nc.vector.tensor_scalar(s2gb[:, DT + i, :], tg[:], 0.5,
                                            None, op0=OP.mult)

        # =============== P3: dbc copy (x-proj accumulated in P2) ============
        nc.scalar.copy(dbc[:], ps3[:, :])
        if debug:
            nc.sync.dma_start(dbg["bc_dbg"][:], dbc[R:R + 2 * N, :])

        # =============== P4: dt = softplus(...) ; dtu = dt*u (Pool) =========
        dt_bf = arena.tile([128, ET, L], BF16, name="dt_bf", tag="s16a")
        dtu = arena.tile([128, ET, L], BF16, name="dtu", tag="s16c")
        # softplus = ln(1 + exp(.)) batched in 2 groups of 4 e-tiles so the
        # exp/ln table loads amortize (exp and ln live in different sets).
        for g in range(2):
            exg = arena.tile([128, 4, L], BF16, name=f"p4_ex{g}", tag="s8a")
            for j in range(4):
                e = g * 4 + j
                ps = psum.tile([128, L], F32, name="p4ps", tag="ps")
                for f in range(L // 512):
                    nc.tensor.matmul(ps[:, f * 512:(f + 1) * 512],
                                     dt_w[:, e * 128:(e + 1) * 128],
                                     dbc[0:R, f * 512:f * 512 + 512],
                                     start=True, stop=True)
                nc.scalar.activation(exg[:, j, :], ps[:, :], AF.Exp,
                                     bias=dt_b[:, e], scale=1.0)
            for j in range(4):
                e = g * 4 + j
                nc.scalar.activation(dt_bf[:, e, :], exg[:, j, :], AF.Ln,
                                     bias=1.0, scale=1.0)
                uk = rot.tile([128, L], BF16, name="p4_uk", tag="u_t", bufs=2)
                nc.sync.dma_start(uk[:], u_dram[:, e, :])
                nc.gpsimd.tensor_tensor(dtu[:, e, :], dt_bf[:, e, :], uk[:],
                                        op=OP.mult)
                if debug:
                    nc.sync.dma_start(
                        dbg["dt_dbg"].rearrange("(a p) m -> p a m", p=128)[:, e, :],
                        dt_bf[:, e, :])

        # =============== P5/P6/P7 per half ==================================
        # BCall layout: rows [0:N) = broadcast B_n, rows [N:2N) = broadcast C_n.
        # Broadcasts are built on PE (ones-row matmul) + ACT copy, keeping the
        # DMA queues free. dBu is batched 4 n's per Pool op to amortize the
        # Q7 launch + semaphore overhead. m_t lags the scan by one iteration
        # so it never reads an hs whose DVE pipe has not drained yet.
        def bcasts(hf, BCb, BCc):
            t0 = hf * HL
            tsl = slice(t0, t0 + HL)
            for n in range(N):
                nc.scalar.dma_start(BCb[:, n, :],
                                    _bcast_row(dbc[R + n:R + n + 1, tsl]))
            for n in range(N):
                nc.sync.dma_start(BCc[:, n, :],
                                  _bcast_row(dbc[R + N + n:R + N + n + 1, tsl]))

        def p567(hf, BCb, BCc, mid_dmas=()):
            t0 = hf * HL
            tsl = slice(t0, t0 + HL)
            y2sb = arena.tile([128, ET, HL], BF16, name=f"y2sb{hf}", tag="s8a")
            for e in range(ET):
                for de, dfn in mid_dmas:
                    if de == e:
                        dfn()
                y2ps = psum.tile([128, HL], F32, name="y2ps", tag="y2ps")
                hs_prev = None
                for n in range(N):
                    if n % 4 == 0:
                        dBu4 = rot.tile([128, 4, HL], BF16, name="dBu4",
                                        tag="cva", bufs=3)
                        nc.gpsimd.tensor_tensor(dBu4[:],
                                                _bcast_mid(dtu[:, e, tsl], 4),
                                                BCb[:, n:n + 4, :], op=OP.mult)
                    dA = rot.tile([128, HL], BF16, name="dA", tag="dA", bufs=2)
                    nc.scalar.activation(dA[:], dt_bf[:, e, tsl], AF.Exp,
                                         bias=0.0, scale=A_sb[:, e, n:n + 1])
                    hs = rot.tile([128, HL], BF16, name="hs", tag="hs", bufs=3)
                    if hf == 0:
                        nc.vector.tensor_tensor_scan(hs[:], dA[:],
                                                     dBu4[:, n % 4, :], 0.0,
                                                     op0=OP.mult, op1=OP.add)
                        nc.scalar.copy(hlast[:, e, n:n + 1], hs[:, HL - 1:HL])
                    else:
                        nc.vector.tensor_tensor_scan(hs[:], dA[:],
                                                     dBu4[:, n % 4, :],
                                                     hlast[:, e, n:n + 1],
                                                     op0=OP.mult, op1=OP.add)
                    if hs_prev is not None:
                        m_t = rot.tile([128, HL], BF16, name="m_t", tag="xs",
                                       bufs=2)
                        nc.vector.tensor_tensor(m_t[:], hs_prev[:],
                                                BCc[:, n - 1, :], op=OP.mult)
                        nc.tensor.matmul(y2ps[:, :], eye_sb[:, :], m_t[:],
                                         start=(n == 1), stop=False)
                    hs_prev = hs
                m_t = rot.tile([128, HL], BF16, name="m_t", tag="xs", bufs=2)
                nc.vector.tensor_tensor(m_t[:], hs_prev[:],
                                        BCc[:, N - 1, :], op=OP.mult)
                nc.tensor.matmul(y2ps[:, :], eye_sb[:, :], m_t[:],
                                 start=False, stop=True)
                uk = rot.tile([128, HL], BF16, name="p6_uk", tag="u_t", bufs=2)
                nc.sync.dma_start(uk[:], u_dram[:, e, tsl])
                szk = rot.tile([128, HL], BF16, name="p6_szk", tag="sz_t",
                               bufs=2)
                nc.scalar.dma_start(szk[:], sz_dram[:, e, tsl])
                ud = rot.tile([128, HL], BF16, name="p6_ud", tag="p6ud", bufs=2)
                nc.vector.scalar_tensor_tensor(ud[:], uk[:], Dsk[:, e],
                                               y2ps[:, :], op0=OP.mult,
                                               op1=OP.add)
                nc.vector.tensor_tensor(y2sb[:, e, :], ud[:], szk[:],
                                        op=OP.mult)
            if debug:
                nc.gpsimd.dma_start(
                    dbg["y2_dbg"].rearrange("(a p) m -> p a m", p=128)[:, :, tsl],
                    y2sb[:])
            for m in range(DT):
                ps = psum.tile([128, HL], F32, name="p7ps", tag="y2ps")
                for k in range(ET):
                    nc.tensor.matmul(ps[:, :], Wd[:, k, m * 128:(m + 1) * 128],
                                     y2sb[:, k, :], start=(k == 0),
                                     stop=(k == ET - 1))
                nc.scalar.copy(p_bf[:, m, tsl], ps[:, :])

        fc1a = wbig.tile([128, DT, FF], BF16, name="fc1a_sb", tag="wslab")
        fc1b = wbig.tile([128, DT, FF], BF16, name="fc1b_sb", tag="wslab")
        _fc1_src = ins["fc1_wT"].rearrange("(a p) m -> p a m", p=128)
        BC0b = arena.tile([128, N, HL], BF16, name="BC0b", tag="s8b")
        BC0c = arena.tile([128, N, HL], BF16, name="BC0c", tag="sCall")
        bcasts(0, BC0b, BC0c)
        p567(0, BC0b, BC0c, mid_dmas=(
            (4, lambda: nc.gpsimd.dma_start(fc1a[:], _fc1_src[:, :, 0:FF])),
            (6, lambda: nc.gpsimd.dma_start(fc1b[:], _fc1_src[:, :, FF:2 * FF])),
        ))

        # ====== early pair exchange: contribute local half0 in global time ==
        for m in range(DT):
            cc0 = rot.tile([128, HL], BF16, name="cc0", tag="ada_sq", bufs=2)
            nc.vector.tensor_scalar(cc0[:], p_bf[:, m, 0:HL], msk[:, 0:1],
                                    None, op0=OP.mult)
            nc.sync.dma_start(cc_in[:, m, 0:HL], cc0[:])
            cc1 = rot.tile([128, HL], BF16, name="cc1", tag="ada_rb", bufs=1)
            nc.vector.tensor_scalar(cc1[:], _rev(p_bf[:, m, 0:HL]), msk[:, 1:2],
                                    None, op0=OP.mult)
            nc.sync.dma_start(cc_in[:, m, HL:L], cc1[:])
        nc.gpsimd.collective_compute(
            "AllReduce", OP.add,
            replica_groups=[[0, 4], [1, 5], [2, 6], [3, 7]],
            ins=[cc_in.opt()], outs=[cc_out.opt()])

        BC1b = arena.tile([128, N, HL], BF16, name="BC1b", tag="s8b")
        BC1c = arena.tile([128, N, HL], BF16, name="BC1c", tag="sCall")
        bcasts(1, BC1b, BC1c)
        p567(1, BC1b, BC1c)
        fc2_w = w_big("fc2_wT", FF, D, q=nc.gpsimd)

        # =============== P9: combine halves + residual ======================
        pg = arena.tile([128, DT, L], BF16, name="pg", tag="s8b")
        nc.sync.dma_start(pg[:], cc_out[:])
        for m in range(DT):
            t0 = rot.tile([128, HL], BF16, name="sel_t0", tag="ada_xr", bufs=2)
            nc.vector.tensor_scalar(t0[:], p_bf[:, m, HL:L], msk[:, 0:1],
                                    None, op0=OP.mult)
            t1 = rot.tile([128, HL], BF16, name="sel_t1", tag="ada_xr", bufs=2)
            nc.vector.scalar_tensor_tensor(t1[:], _rev(p_bf[:, m, HL:L]),
                                           msk[:, 1:2], t0[:],
                                           op0=OP.mult, op1=OP.add)
            t2 = rot.tile([128, HL], BF16, name="sel_t2", tag="ada_xr", bufs=2)
            nc.vector.scalar_tensor_tensor(t2[:], pg[:, m, HL:L], msk[:, 0:1],
                                           t1[:], op0=OP.mult, op1=OP.add)
            t3 = rot.tile([128, HL], BF16, name="sel_t3", tag="ada_xr", bufs=2)
            nc.vector.scalar_tensor_tensor(t3[:], pg[:, m, 0:HL], msk[:, 1:2],
                                           t2[:], op0=OP.mult, op1=OP.add)
            nc.vector.scalar_tensor_tensor(x1[:, m, :], t3[:], ls1[:, m],
                                           x_half[:, m, :], op0=OP.mult,
                                           op1=OP.add)
        if debug:
            x1d = rot.tile([128, DT, HL], F32, name="x1d", tag="x1d", bufs=1)
            for m in range(DT):
                nc.vector.tensor_copy(x1d[:, m, :], x1[:, m, :])
            nc.sync.dma_start(
                dbg["x1_dbg"].rearrange("(a p) m -> p a m", p=128), x1d[:])

        # =============== P10: ada_norm 2 (precomputed gamma/beta) ===========
        ms2 = psum.tile([1, HL], F32, name="ms2", tag="msps", bufs=1)
        for i in range(DT):
            sq = rot.tile([128, HL], BF16, name="p10_sq", tag="ada_sq", bufs=2)
            nc.vector.tensor_tensor(sq[:], x1[:, i, :], x1[:, i, :], op=OP.mult)
            nc.tensor.matmul(ms2[:, :], ones_col[:], sq[:],
                             start=(i == 0), stop=(i == DT - 1))
        lnm2 = rot.tile([1, HL], BF16, name="lnm2", tag="ada_lnm", bufs=1)
        nc.scalar.activation(lnm2[:], ms2[:], AF.Ln, bias=eps_t[:], scale=1.0 / D)
        rinv2 = rot.tile([1, HL], BF16, name="rinv2", tag="ada_rinv", bufs=1)
        nc.scalar.activation(rinv2[:], lnm2[:], AF.Exp, bias=0.0, scale=-0.5)
        rb2 = rot.tile([128, HL], BF16, name="rb2", tag="ada_rb", bufs=1)
        nc.sync.dma_start(rb2[:], _bcast_row(rinv2[0:1, :]))
        h2 = arena.tile([128, DT, HL], BF16, name="h2", tag="sCall")
        for i in range(DT):
            xr = rot.tile([128, HL], BF16, name="p10_xr", tag="ada_xr", bufs=2)
            nc.vector.tensor_tensor(xr[:], x1[:, i, :], rb2[:, :], op=OP.mult)
            hp = rot.tile([128, HL], BF16, name="p10_hp", tag="ada_xr", bufs=2)
            nc.vector.tensor_tensor(hp[:], xr[:], s2gb[:, i, :], op=OP.mult)
            nc.vector.tensor_tensor(h2[:, i, :], hp[:], s2gb[:, DT + i, :],
                                    op=OP.add)

        # =============== P11: SwiGLU FFN ====================================
        sgf = arena.tile([128, FT, HL], BF16, name="sgf", tag="s16a")
        gv = arena.tile([128, FT, HL], BF16, name="gv", tag="s16c")
        for m in range(2 * FT):
            ps = psum.tile([128, HL], F32, name="p11ps", tag="y2ps")
            fw = fc1a if m < FT else fc1b
            mc = m if m < FT else m - FT
            for k in range(DT):
                nc.tensor.matmul(ps[:, :], fw[:, k, mc * 128:(mc + 1) * 128],
                                 h2[:, k, :], start=(k == 0), stop=(k == DT - 1))
            if m < FT:
                nc.scalar.activation(sgf[:, m, :], ps[:, :], AF.Silu,
                                     bias=fc1_b[:, m], scale=1.0)
            else:
                vv = rot.tile([128, HL], BF16, name="p11_vv", tag="ada_tg", bufs=2)
                nc.scalar.activation(vv[:], ps[:, :], AF.Identity,
                                     bias=fc1_b[:, m], scale=1.0)
                nc.vector.tensor_tensor(gv[:, m - FT, :], sgf[:, m - FT, :],
                                        vv[:], op=OP.mult)
        out_sb = arena.tile([128, DT, HL], F32, name="out_sb", tag="s8x")
        for m in range(DT):
            ps = psum.tile([128, HL], F32, name="p12ps", tag="y2ps")
            for k in range(FT):
                nc.tensor.matmul(ps[:, :], fc2_w[:, k, m * 128:(m + 1) * 128],
                                 gv[:, k, :], start=(k == 0), stop=False)
            nc.tensor.matmul(ps[:, :], c2row[:, m * 128:(m + 1) * 128],
                             ones_row[:], start=False, stop=True)
            nc.vector.scalar_tensor_tensor(out_sb[:, m, :], ps[:, :], ls2[:, m],
                                           x1[:, m, :], op0=OP.mult, op1=OP.add)
        nc.sync.dma_start(out_ap.rearrange("(a p) m -> p a m", p=128), out_sb[:])

    nc.compile()
    return nc, dbg


def _prep_inputs(inputs):
    """Host-side: per-core input dicts."""
    f32 = np.float32
    bf = ml_dtypes.bfloat16
    x = np.asarray(inputs["x"], f32)
    pn = np.asarray(inputs["phys_norm"], f32)
    blk_w = np.asarray(inputs["blk_w"], f32)
    ls1v = np.asarray(inputs["ls1"], f32)
    blk_b = np.asarray(inputs["blk_b"], f32)
    in_maps = []
    for c in range(NCORES):
        d, b = c // 4, c % 4
        xb = x[b] if d == 0 else x[b, ::-1]
        pnb = pn[b] if d == 0 else pn[b, ::-1]
        osl = slice(HL, L) if d == 0 else slice(0, HL)  # my output half (global)
        xh = x[b, osl] + (ls1v * blk_b)[None, :]
        Wd = blk_w[:, d * D:(d + 1) * D] @ np.asarray(inputs["m_out_w"][d], f32)
        m = {
            "xhT": np.ascontiguousarray(xh.T),
            "xbT": np.ascontiguousarray(xb.T).astype(bf),
            "pnT": np.ascontiguousarray(pnb.T).astype(bf),
            "pnoT": np.ascontiguousarray(pn[b, osl].T).astype(bf),
            "dirmask": np.tile(np.array([[1.0 - d, float(d)]], f32), (128, 1)),
            "eye": np.eye(128, dtype=f32).astype(bf),
            "in_wT": np.ascontiguousarray(inputs["m_in_w"][d].T).astype(bf),
            "conv_w": np.asarray(inputs["m_conv_w"][d], f32),
            "conv_b": np.asarray(inputs["m_conv_b"][d], f32).reshape(ED, 1),
            "xp_wT": np.ascontiguousarray(inputs["m_xproj_w"][d].T).astype(bf),
            "dt_wT": np.ascontiguousarray(inputs["m_dt_w"][d].T).astype(bf),
            "dt_b": np.asarray(inputs["m_dt_b"][d], f32).reshape(ED, 1),
            "A": (-np.exp(np.asarray(inputs["m_A_log"][d], f32))),
            "Dsk": np.asarray(inputs["m_D"][d], f32).reshape(ED, 1),
            "WdT": np.ascontiguousarray(Wd.T).astype(bf),
            "ls1": ls1v.reshape(D, 1),
            "fc1_wT": np.ascontiguousarray(inputs["fc1_w"].T).astype(bf),
            "fc1_b": np.asarray(inputs["fc1_b"], f32).reshape(2 * FF, 1),
            "fc2_wT": np.ascontiguousarray(inputs["fc2_w"].T).astype(bf),
            "ls2": np.asarray(inputs["ls2"], f32).reshape(D, 1),
            "c2T": np.asarray(inputs["fc2_b"], f32).reshape(1, D).astype(bf),
        }
        for p in ("n1", "n2"):
            m[p + "_w1T"] = np.ascontiguousarray(inputs[p + "_w1"].T).astype(bf)
            m[p + "_b1"] = np.asarray(inputs[p + "_b1"], f32).reshape(2 * D, 1)
            m[p + "_w2T"] = np.ascontiguousarray(inputs[p + "_w2"].T).astype(bf)
            m[p + "_b2"] = np.asarray(inputs[p + "_b2"], f32).reshape(2 * D, 1)
            m[p + "_sc"] = np.asarray(inputs[p + "_scale"], f32).reshape(D, 1)
        in_maps.append(m)
    return in_maps


def run(inputs, debug=False, trace=False):
    key = ("dbg" if debug else "lean")
    if key not in _cache:
        _cache[key] = build(debug=debug)
    nc, dbg = _cache[key]
    in_maps = _prep_inputs(inputs)
    res = run_bass_kernel_spmd(nc, in_maps, core_ids=list(range(NCORES)),
                               trace=trace)
    out = np.zeros((B, L, D), np.float32)
    for c in range(NCORES):
        d, b = c // 4, c % 4
        o = res.results[c]["out"]  # [D, HL], global order, my half
        if d == 0:
            out[b, HL:L] = o.T
        else:
            out[b, 0:HL] = o.T
    return out, res


def kernel(**inputs):
    out, _ = run(inputs, debug=False, trace=False)
    return out


# revision 13
# speedup vs baseline: 1.0344x; 1.0338x over previous
# Trainium2 Bass kernel for nn_BiMambaLayer (BiMamba block: AdaRMSNorm ->
# bidirectional Mamba -> out-proj residual -> AdaRMSNorm -> SwiGLU FFN).
#
# Sharding: 8 cores = 2 directions x 4 batches (core c: dir=c//4, b=c%4).
# Each core runs one direction of one sequence in its local time order
# (dir-1 cores receive time-flipped inputs, so the SPMD program is identical).
# Each core OUTPUTS the global-time half that equals its LOCAL second half,
# so the pair exchange of the mamba branch (the local first halves) overlaps
# with the second half's scan: after p7(0) each core contributes its local
# half0 (canonicalized to global time) to a pair AllReduce, keeps scanning,
# and only at the tail adds its own local half1 to the received partner half.
#
# Engine split for the scan phase (p5), per (e,n):
#   ACT:  dA = exp(A[e,n] * dt)           (exp table resident all phase)
#   Pool: dBu = dtu * B_n                 (gpsimd tensor_tensor)
#   DVE:  hs  = tensor_tensor_scan(dA, dBu)
#   DVE:  m_t = hs * C_n
#   PE:   y2_psum += I @ m_t              (accumulates over n per e-tile)
# Layout: feature-major everywhere [feature on partitions, time on free dim].
import numpy as np
import ml_dtypes

try:
    import ntff_hook_shim  # noqa: F401  (optional, enables trace in dev)
    ntff_hook_shim.install()
except Exception:
    pass

import concourse.bass as bass
import concourse.tile as tile
from concourse import bacc, mybir
from concourse.bass_utils import run_bass_kernel_spmd
from contextlib import ExitStack

F32 = mybir.dt.float32
BF16 = mybir.dt.bfloat16
AF = mybir.ActivationFunctionType
OP = mybir.AluOpType

D = 512          # d_model
ED = 1024        # d_inner
N = 16           # d_state
R = 32           # dt_rank
DC = 4           # d_conv
FF = 1536        # d_ff
B, L = 4, 1024
EPS = 1e-6
HL = L // 2      # tokens per half / per core in the FFN phase
NCORES = 8
ET = ED // 128   # 8 e-tiles
DT = D // 128    # 4 d-tiles
FT = FF // 128   # 12

_cache = {}


def _rev(ap):
    """Reverse the (last) free dim of a 2D AP."""
    a = list(ap.ap)
    assert len(a) == 2
    stride, n = a[1]
    return bass.AP(tensor=ap.tensor, offset=ap.offset + stride * (n - 1),
                  ap=[a[0], [-stride, n]])


def _bcast_mid(ap2d, k):
    """Insert a stride-0 middle free dim of size k into a 2D AP."""
    a = list(ap2d.ap)
    return bass.AP(tensor=ap2d.tensor, offset=ap2d.offset,
                  ap=[a[0], [0, k], a[1]])


def _bcast_row(src_row):
    """AP that reads a [1, F] SBUF row 128x (replication via stride-0 free dim)."""
    a = list(src_row.ap)
    return bass.AP(tensor=src_row.tensor, offset=src_row.offset,
                  ap=[a[0], [0, 128], a[1]])


def build(debug=False):
    nc = bacc.Bacc("TRN2", target_bir_lowering=False, debug=False,
                   num_devices=NCORES)

    def din(name, shape, dt=F32):
        return nc.dram_tensor(name, shape, dt, kind="ExternalInput").ap()

    ins = {}
    ins["xhT"] = din("xhT", [D, HL])               # x at MY global half (+ls1*blk_b)
    ins["xbT"] = din("xbT", [D, L], BF16)          # x[b].T local time (flipped dir1)
    ins["pnT"] = din("pnT", [2, L], BF16)          # phys_norm[b].T local time
    ins["pnoT"] = din("pnoT", [2, HL], BF16)       # phys_norm at MY half, global
    ins["dirmask"] = din("dirmask", [128, 2])      # col0=1-dir, col1=dir
    ins["eye"] = din("eye", [128, 128], BF16)
    for p in ("n1", "n2"):
        ins[p + "_w1T"] = din(p + "_w1T", [2, 2 * D], BF16)
        ins[p + "_b1"] = din(p + "_b1", [2 * D, 1])
        ins[p + "_w2T"] = din(p + "_w2T", [2 * D, 2 * D], BF16)
        ins[p + "_b2"] = din(p + "_b2", [2 * D, 1])
        ins[p + "_sc"] = din(p + "_sc", [D, 1])
    ins["in_wT"] = din("in_wT", [D, 2 * ED], BF16)
    ins["conv_w"] = din("conv_w", [ED, DC])
    ins["conv_b"] = din("conv_b", [ED, 1])
    ins["xp_wT"] = din("xp_wT", [ED, R + 2 * N], BF16)
    ins["dt_wT"] = din("dt_wT", [R, ED], BF16)
    ins["dt_b"] = din("dt_b", [ED, 1])
    ins["A"] = din("A", [ED, N])
    ins["Dsk"] = din("Dsk", [ED, 1])
    ins["WdT"] = din("WdT", [ED, D], BF16)         # (blk_half_dir @ out_w_dir).T
    ins["ls1"] = din("ls1", [D, 1])
    ins["fc1_wT"] = din("fc1_wT", [D, 2 * FF], BF16)
    ins["fc1_b"] = din("fc1_b", [2 * FF, 1])
    ins["fc2_wT"] = din("fc2_wT", [FF, D], BF16)
    ins["ls2"] = din("ls2", [D, 1])
    ins["c2T"] = din("c2T", [1, D], BF16)          # fc2_b as a row

    out_ap = nc.dram_tensor("out", [D, HL], F32, kind="ExternalOutput").ap()
    dbg = {}
    if debug:
        def dout(name, shape, dt=BF16):
            dbg[name] = nc.dram_tensor(name, shape, dt, kind="ExternalOutput").ap()
        dout("h_dbg", [D, L])
        dout("u_dbg", [ED, L])
        dout("sz_dbg", [ED, L])
        dout("dt_dbg", [ED, L])
        dout("bc_dbg", [2 * N, L])
        dout("y2_dbg", [ED, L])
        dout("x1_dbg", [D, HL], F32)

    with tile.TileContext(nc) as tc, ExitStack() as ctx:
        wpool = ctx.enter_context(tc.tile_pool(name="weights", bufs=1))
        wbig = ctx.enter_context(tc.tile_pool(name="wbig", bufs=2))
        arena = ctx.enter_context(tc.tile_pool(name="arena", bufs=1))
        rot = ctx.enter_context(tc.tile_pool(name="rot", bufs=2))
        psum = ctx.enter_context(tc.tile_pool(name="psum", bufs=2, space="PSUM"))
        dram = ctx.enter_context(tc.tile_pool(name="dram", bufs=1, space="DRAM"))

        _dma_rr = [nc.sync, nc.scalar]
        _rr = [0]

        def _wdma(out, in_):
            _dma_rr[_rr[0] % 2].dma_start(out, in_)
            _rr[0] += 1

        def w_big(name, K, M, src=None, q=None):
            t = wbig.tile([128, K // 128, M], BF16, name=name + "_sb", tag="wslab")
            if src is None:
                src = ins[name].rearrange("(a p) m -> p a m", p=128)
            if q is None:
                _wdma(t[:], src)
            else:
                q.dma_start(t[:], src)
            return t

        def w_perm(name, K, M, dt=BF16):
            t = wpool.tile([128, K // 128, M], dt, name=name + "_sb")
            _wdma(t[:], ins[name].rearrange("(a p) m -> p a m", p=128))
            return t

        def w_vec(name, K, dt=F32):
            t = wpool.tile([128, K // 128, 1], dt, name=name + "_sb")
            _wdma(t[:], ins[name].rearrange("(a p) o -> p a o", p=128))
            return t

        # ---- permanent small weights ----
        pn_sb = wpool.tile([2, L], BF16, name="pn_sb")
        nc.sync.dma_start(pn_sb[:], ins["pnT"][:])
        pno_sb = wpool.tile([2, HL], BF16, name="pno_sb")
        nc.sync.dma_start(pno_sb[:], ins["pnoT"][:])
        msk = wpool.tile([128, 2], F32, name="msk_sb")
        nc.sync.dma_start(msk[:], ins["dirmask"][:])
        eye_sb = wpool.tile([128, 128], BF16, name="eye_sb")
        nc.sync.dma_start(eye_sb[:], ins["eye"][:])
        n1_w1 = wpool.tile([2, 2 * D], BF16, name="n1_w1_sb")
        nc.sync.dma_start(n1_w1[:], ins["n1_w1T"][:])
        n2_w1 = wpool.tile([2, 2 * D], BF16, name="n2_w1_sb")
        nc.sync.dma_start(n2_w1[:], ins["n2_w1T"][:])
        n1_b1 = w_vec("n1_b1", 2 * D)
        n1_b2 = w_vec("n1_b2", 2 * D)
        n1_sc = w_vec("n1_sc", D)
        n2_b1 = w_vec("n2_b1", 2 * D)
        n2_b2 = w_vec("n2_b2", 2 * D)
        n2_sc = w_vec("n2_sc", D)
        conv_w = wpool.tile([128, ET, DC], F32, name="conv_w_sb")
        nc.sync.dma_start(conv_w[:], ins["conv_w"].rearrange("(a p) m -> p a m", p=128))
        conv_b = w_vec("conv_b", ED)
        xp_w = w_perm("xp_wT", ED, R + 2 * N)
        dt_w = wpool.tile([R, ED], BF16, name="dt_w_sb")
        nc.sync.dma_start(dt_w[:], ins["dt_wT"][:])
        dt_b = w_vec("dt_b", ED)
        A_sb = wpool.tile([128, ET, N], F32, name="A_sb")
        nc.sync.dma_start(A_sb[:], ins["A"].rearrange("(a p) m -> p a m", p=128))
        Dsk = w_vec("Dsk", ED)
        Wd = w_perm("WdT", ED, D)
        ls1 = w_vec("ls1", D)
        ls2 = w_vec("ls2", D)
        fc1_b = w_vec("fc1_b", 2 * FF)

        ones_col = wpool.tile([128, 1], BF16, name="ones_col")
        nc.vector.memset(ones_col[:], 1.0)
        ones_row = wpool.tile([1, HL], BF16, name="ones_row")
        nc.vector.memset(ones_row[:], 1.0)
        c2row = wpool.tile([1, D], BF16, name="c2row")
        nc.sync.dma_start(c2row[:], ins["c2T"][:])
        eps_t = wpool.tile([1, 1], F32, name="eps_t")
        nc.vector.memset(eps_t[:], EPS)

        # persistent / tag-shared big tiles
        p_bf = arena.tile([128, DT, L], BF16, name="p_bf", tag="p_bf")
        hlast = arena.tile([128, ET, N], F32, name="hlast", tag="hlast")
        x_half = arena.tile([128, DT, HL], F32, name="x_half", tag="s8x")
        nc.sync.dma_start(x_half[:],
                          ins["xhT"].rearrange("(a p) m -> p a m", p=128))
        dbc = arena.tile([R + 2 * N, L], BF16, name="dbc", tag="dbc")
        x1 = arena.tile([128, DT, HL], BF16, name="x1", tag="x1")
        u_dram = dram.tile([128, ET, L], BF16, name="u_dram")
        sz_dram = dram.tile([128, ET, L], BF16, name="sz_dram")
        cc_in = dram.tile([128, DT, L], BF16, name="cc_in")
        cc_out = dram.tile([128, DT, L], BF16, name="cc_out")

        n1_w2 = w_big("n1_w2T", 2 * D, 2 * D)
        in_w = w_big("in_wT", D, 2 * ED)

        # =============== P1: ada_norm 1 over full L (feature-major) =========
        xb = arena.tile([128, DT, L], BF16, name="xb", tag="s16c")
        nc.sync.dma_start(xb[:], ins["xbT"].rearrange("(a p) m -> p a m", p=128))
        h1 = arena.tile([128, DT, L], BF16, name="h1", tag="s8a")

        ms_ps = psum.tile([1, L], F32, name="ms_ps", tag="msps", bufs=1)
        for i in range(DT):
            sq = rot.tile([128, L], BF16, name="p1_sq", tag="ada_sq", bufs=2)
            nc.vector.tensor_tensor(sq[:], xb[:, i, :], xb[:, i, :], op=OP.mult)
            for f in range(L // 512):
                nc.tensor.matmul(ms_ps[:, f * 512:(f + 1) * 512],
                                 ones_col[:], sq[:, f * 512:(f + 1) * 512],
                                 start=(i == 0), stop=(i == DT - 1))
        lnm = rot.tile([1, L], BF16, name="p1_lnm", tag="ada_lnm", bufs=1)
        nc.scalar.activation(lnm[:], ms_ps[:], AF.Ln, bias=eps_t[:], scale=1.0 / D)
        rinv = rot.tile([1, L], BF16, name="p1_rinv", tag="ada_rinv", bufs=1)
        nc.scalar.activation(rinv[:], lnm[:], AF.Exp, bias=0.0, scale=-0.5)
        rb = rot.tile([128, L], BF16, name="p1_rb", tag="ada_rb", bufs=1)
        nc.sync.dma_start(rb[:], _bcast_row(rinv[0:1, :]))

        # cond MLP 1 (silu/tanh table window)
        sg = arena.tile([128, 2 * DT, L], BF16, name="p1_sg", tag="s16a")
        for m in range(2 * DT):
            ps = psum.tile([128, L], F32, name="p1_ps1", tag="ps")
            for f in range(L // 512):
                nc.tensor.matmul(ps[:, f * 512:(f + 1) * 512],
                                 n1_w1[:, m * 128:(m + 1) * 128],
                                 pn_sb[:, f * 512:f * 512 + 512],
                                 start=True, stop=True)
            nc.scalar.activation(sg[:, m, :], ps[:, :], AF.Silu,
                                 bias=n1_b1[:, m], scale=1.0)
        for i in range(DT):
            tgp = []
            for mm in (i, DT + i):
                tg = rot.tile([128, L], BF16, name="p1_tg", tag="ada_tg", bufs=2)
                ps = psum.tile([128, L], F32, name="p1_ps2", tag="ps")
                for f in range(L // 512):
                    for k in range(2 * DT):
                        nc.tensor.matmul(
                            ps[:, f * 512:(f + 1) * 512],
                            n1_w2[:, k, mm * 128:(mm + 1) * 128],
                            sg[:, k, f * 512:f * 512 + 512],
                            start=(k == 0), stop=(k == 2 * DT - 1))
                nc.scalar.activation(tg[:], ps[:, :], AF.Tanh,
                                     bias=n1_b2[:, mm], scale=1.0)
                tgp.append(tg)
            s1 = rot.tile([128, L], BF16, name="p1_s1", tag="sz_t", bufs=2)
            nc.vector.tensor_scalar(s1[:], tgp[0][:], 0.5, n1_sc[:, i],
                                    op0=OP.mult, op1=OP.add)
            xr = rot.tile([128, L], BF16, name="p1_xr", tag="ada_xr", bufs=2)
            nc.vector.tensor_tensor(xr[:], xb[:, i, :], rb[:, :], op=OP.mult)
            hp = rot.tile([128, L], BF16, name="p1_hp", tag="ada_xr", bufs=2)
            nc.vector.tensor_tensor(hp[:], xr[:], s1[:], op=OP.mult)
            nc.vector.scalar_tensor_tensor(h1[:, i, :], tgp[1][:], 0.5, hp[:],
                                           op0=OP.mult, op1=OP.add)
        if debug:
            nc.sync.dma_start(dbg["h_dbg"].rearrange("(a p) m -> p a m", p=128),
                              h1[:])

        # =============== P2: in-proj, causal conv, silu -> u, sz ============
        # x-proj (old p3) folded in: ps3 accumulates xp_w[m] @ u[m] as each
        # u-tile is produced, so dbc is ready right after the m-loop.
        ps3 = psum.tile([R + 2 * N, L], F32, name="p3ps", tag="msps", bufs=1)
        for m in range(ET):
            ps = psum.tile([128, L], F32, name="p2ps", tag="ps")
            for f in range(L // 512):
                for k in range(DT):
                    nc.tensor.matmul(ps[:, f * 512:(f + 1) * 512],
                                     in_w[:, k, m * 128:(m + 1) * 128],
                                     h1[:, k, f * 512:f * 512 + 512],
                                     start=(k == 0), stop=(k == DT - 1))
            xs = rot.tile([128, DC - 1 + L], BF16, name="p2_xs", tag="xs", bufs=2)
            nc.vector.memset(xs[:, 0:DC - 1], 0.0)
            nc.scalar.copy(xs[:, DC - 1:], ps[:, :])
            acc = rot.tile([128, L], BF16, name="p2_acc", tag="u_t", bufs=2)
            nc.vector.tensor_scalar(acc[:], xs[:, 0:L], conv_w[:, m, 0:1],
                                    None, op0=OP.mult)
            for k in range(1, DC):
                acc2 = rot.tile([128, L], BF16, name="p2_acc2", tag="cva", bufs=3)
                nc.vector.scalar_tensor_tensor(acc2[:], xs[:, k:k + L],
                                               conv_w[:, m, k:k + 1], acc[:],
                                               op0=OP.mult, op1=OP.add)
                acc = acc2
            u_t = rot.tile([128, L], BF16, name="p2_u", tag="u_t", bufs=2)
            nc.scalar.activation(u_t[:], acc[:], AF.Silu,
                                 bias=conv_b[:, m], scale=1.0)
            nc.sync.dma_start(u_dram[:, m, :], u_t[:])
            for f in range(L // 512):
                nc.tensor.matmul(ps3[:, f * 512:(f + 1) * 512],
                                 xp_w[:, m, :], u_t[:, f * 512:f * 512 + 512],
                                 start=(m == 0), stop=(m == ET - 1))
            ps2 = psum.tile([128, L], F32, name="p2ps2", tag="ps")
            for f in range(L // 512):
                for k in range(DT):
                    nc.tensor.matmul(ps2[:, f * 512:(f + 1) * 512],
                                     in_w[:, k, (ET + m) * 128:(ET + m + 1) * 128],
                                     h1[:, k, f * 512:f * 512 + 512],
                                     start=(k == 0), stop=(k == DT - 1))
            sz_t = rot.tile([128, L], BF16, name="p2_sz", tag="sz_t", bufs=2)
            nc.scalar.activation(sz_t[:], ps2[:, :], AF.Silu)
            nc.scalar.dma_start(sz_dram[:, m, :], sz_t[:])
        if debug:
            nc.gpsimd.dma_start(
                dbg["u_dbg"].rearrange("(a p) m -> p a m", p=128), u_dram[:])
            nc.gpsimd.dma_start(
                dbg["sz_dbg"].rearrange("(a p) m -> p a m", p=128), sz_dram[:])

        # =============== cond MLP 2 precompute (pn_out only; silu window) ===
        n2_w2 = w_big("n2_w2T", 2 * D, 2 * D)
        s2gb = arena.tile([128, 2 * DT, HL], BF16, name="s2gb", tag="s2gb")
        sg2 = arena.tile([128, 2 * DT, HL], BF16, name="sg2c", tag="s8b")
        for m in range(2 * DT):
            ps = psum.tile([128, HL], F32, name="c2ps1", tag="y2ps")
            nc.tensor.matmul(ps[:, :], n2_w1[:, m * 128:(m + 1) * 128],
                             pno_sb[:, :], start=True, stop=True)
            nc.scalar.activation(sg2[:, m, :], ps[:, :], AF.Silu,
                                 bias=n2_b1[:, m], scale=1.0)
        for i in range(DT):
            for j, mm in enumerate((i, DT + i)):
                ps = psum.tile([128, HL], F32, name="c2ps2", tag="y2ps")
                for k in range(2 * DT):
                    nc.tensor.matmul(ps[:, :], n2_w2[:, k, mm * 128:(mm + 1) * 128],
                                     sg2[:, k, :], start=(k == 0),
                                     stop=(k == 2 * DT - 1))
                tg = rot.tile([128, HL], BF16, name="c2_tg", tag="ada_tg", bufs=2)
                nc.scalar.activation(tg[:], ps[:, :], AF.Tanh,
                                     bias=n2_b2[:, mm], scale=1.0)
                if j == 0:
                    nc.vector.tensor_scalar(s2gb[:, i, :], tg[:], 0.5,
                                            n2_sc[:, i], op0=OP.mult, op1=OP.add)
                else:
                    nc.vector.tensor_scalar(s2gb[:, DT + i, :], tg[:], 0.5,
                                            None, op0=OP.mult)

        # =============== P3: dbc copy (x-proj accumulated in P2) ============
        nc.scalar.copy(dbc[:], ps3[:, :])
        if debug:
            nc.sync.dma_start(dbg["bc_dbg"][:], dbc[R:R + 2 * N, :])

        # =============== P4: dt = softplus(...) ; dtu = dt*u (Pool) =========
        dt_bf = arena.tile([128, ET, L], BF16, name="dt_bf", tag="s16a")
        dtu = arena.tile([128, ET, L], BF16, name="dtu", tag="s16c")
        # softplus = ln(1 + exp(.)) batched in 2 groups of 4 e-tiles so the
        # exp/ln table loads amortize (exp and ln live in different sets).
        for g in range(2):
            exg = arena.tile([128, 4, L], BF16, name=f"p4_ex{g}", tag="s8a")
            for j in range(4):
                e = g * 4 + j
                ps = psum.tile([128, L], F32, name="p4ps", tag="ps")
                for f in range(L // 512):
                    nc.tensor.matmul(ps[:, f * 512:(f + 1) * 512],
                                     dt_w[:, e * 128:(e + 1) * 128],
                                     dbc[0:R, f * 512:f * 512 + 512],
                                     start=True, stop=True)
                nc.scalar.activation(exg[:, j, :], ps[:, :], AF.Exp,
                                     bias=dt_b[:, e], scale=1.0)
            for j in range(4):
                e = g * 4 + j
                nc.scalar.activation(dt_bf[:, e, :], exg[:, j, :], AF.Ln,
                                     bias=1.0, scale=1.0)
                uk = rot.tile([128, L], BF16, name="p4_uk", tag="u_t", bufs=2)
                nc.sync.dma_start(uk[:], u_dram[:, e, :])
                nc.gpsimd.tensor_tensor(dtu[:, e, :], dt_bf[:, e, :], uk[:],
                                        op=OP.mult)
                if debug:
                    nc.sync.dma_start(
                        dbg["dt_dbg"].rearrange("(a p) m -> p a m", p=128)[:, e, :],
                        dt_bf[:, e, :])

        # =============== P5/P6/P7 per half ==================================
        # BCall layout: rows [0:N) = broadcast B_n, rows [N:2N) = broadcast C_n.
        # Broadcasts are built on PE (ones-row matmul) + ACT copy, keeping the
        # DMA queues free. dBu is batched 4 n's per Pool op to amortize the
        # Q7 launch + semaphore overhead. m_t lags the scan by one iteration
        # so it never reads an hs whose DVE pipe has not drained yet.
        def bcasts(hf, BCb, BCc):
            t0 = hf * HL
            tsl = slice(t0, t0 + HL)
            for n in range(N):
                nc.scalar.dma_start(BCb[:, n, :],
                                    _bcast_row(dbc[R + n:R + n + 1, tsl]))
            for n in range(N):
                nc.sync.dma_start(BCc[:, n, :],
                                  _bcast_row(dbc[R + N + n:R + N + n + 1, tsl]))

        def p567(hf, BCb, BCc, mid_dmas=()):
            t0 = hf * HL
            tsl = slice(t0, t0 + HL)
            y2sb = arena.tile([128, ET, HL], BF16, name=f"y2sb{hf}", tag="s8a")

            def p6(e, y2ps_e):
                uk = rot.tile([128, HL], BF16, name="p6_uk", tag="u_t", bufs=2)
                nc.sync.dma_start(uk[:], u_dram[:, e, tsl])
                szk = rot.tile([128, HL], BF16, name="p6_szk", tag="sz_t",
                               bufs=2)
                nc.scalar.dma_start(szk[:], sz_dram[:, e, tsl])
                ud = rot.tile([128, HL], BF16, name="p6_ud", tag="p6ud", bufs=2)
                nc.vector.scalar_tensor_tensor(ud[:], uk[:], Dsk[:, e],
                                               y2ps_e[:, :], op0=OP.mult,
                                               op1=OP.add)
                nc.vector.tensor_tensor(y2sb[:, e, :], ud[:], szk[:],
                                        op=OP.mult)

            y2ps_prev = None
            for e in range(ET):
                for de, dfn in mid_dmas:
                    if de == e:
                        dfn()
                y2ps = psum.tile([128, HL], F32, name="y2ps", tag="y2ps")
                hs_prev = None
                for n in range(N):
                    if n % 4 == 0:
                        dBu4 = rot.tile([128, 4, HL], BF16, name="dBu4",
                                        tag="cva", bufs=3)
                        nc.gpsimd.tensor_tensor(dBu4[:],
                                                _bcast_mid(dtu[:, e, tsl], 4),
                                                BCb[:, n:n + 4, :], op=OP.mult)
                    dA = rot.tile([128, HL], BF16, name="dA", tag="dA", bufs=2)
                    nc.scalar.activation(dA[:], dt_bf[:, e, tsl], AF.Exp,
                                         bias=0.0, scale=A_sb[:, e, n:n + 1])
                    hs = rot.tile([128, HL], BF16, name="hs", tag="hs", bufs=3)
                    if hf == 0:
                        nc.vector.tensor_tensor_scan(hs[:], dA[:],
                                                     dBu4[:, n % 4, :], 0.0,
                                                     op0=OP.mult, op1=OP.add)
                        nc.scalar.copy(hlast[:, e, n:n + 1], hs[:, HL - 1:HL])
                    else:
                        nc.vector.tensor_tensor_scan(hs[:], dA[:],
                                                     dBu4[:, n % 4, :],
                                                     hlast[:, e, n:n + 1],
                                                     op0=OP.mult, op1=OP.add)
                    if hs_prev is not None:
                        m_t = rot.tile([128, HL], BF16, name="m_t", tag="xs",
                                       bufs=2)
                        nc.vector.tensor_tensor(m_t[:], hs_prev[:],
                                                BCc[:, n - 1, :], op=OP.mult)
                        nc.tensor.matmul(y2ps[:, :], eye_sb[:, :], m_t[:],
                                         start=(n == 1), stop=False)
                    hs_prev = hs
                m_t = rot.tile([128, HL], BF16, name="m_t", tag="xs", bufs=2)
                nc.vector.tensor_tensor(m_t[:], hs_prev[:],
                                        BCc[:, N - 1, :], op=OP.mult)
                nc.tensor.matmul(y2ps[:, :], eye_sb[:, :], m_t[:],
                                 start=False, stop=True)
                if e > 0:
                    p6(e - 1, y2ps_prev)
                y2ps_prev = y2ps
            p6(ET - 1, y2ps_prev)
            for m in range(DT):
                ps = psum.tile([128, HL], F32, name="p7ps", tag="y2ps")
                for k in range(ET):
                    nc.tensor.matmul(ps[:, :], Wd[:, k, m * 128:(m + 1) * 128],
                                     y2sb[:, k, :], start=(k == 0),
                                     stop=(k == ET - 1))
                nc.scalar.copy(p_bf[:, m, tsl], ps[:, :])

        fc1a = wbig.tile([128, DT, FF], BF16, name="fc1a_sb", tag="wslab")
        fc1b = wbig.tile([128, DT, FF], BF16, name="fc1b_sb", tag="wslab")
        _fc1_src = ins["fc1_wT"].rearrange("(a p) m -> p a m", p=128)
        BCb = arena.tile([128, N, HL], BF16, name="BCb", tag="s8b")
        BCc = arena.tile([128, N, HL], BF16, name="BCc", tag="sCall")
        bcasts(0, BCb, BCc)
        p567(0, BCb, BCc, mid_dmas=(
            (4, lambda: nc.gpsimd.dma_start(fc1a[:], _fc1_src[:, :, 0:FF])),
            (6, lambda: nc.gpsimd.dma_start(fc1b[:], _fc1_src[:, :, FF:2 * FF])),
        ))

        # ====== early pair exchange: contribute local half0 in global time ==
        for m in range(DT):
            cc0 = rot.tile([128, HL], BF16, name="cc0", tag="ada_sq", bufs=2)
            nc.vector.tensor_scalar(cc0[:], p_bf[:, m, 0:HL], msk[:, 0:1],
                                    None, op0=OP.mult)
            nc.sync.dma_start(cc_in[:, m, 0:HL], cc0[:])
            cc1 = rot.tile([128, HL], BF16, name="cc1", tag="ada_rb", bufs=1)
            nc.vector.tensor_scalar(cc1[:], _rev(p_bf[:, m, 0:HL]), msk[:, 1:2],
                                    None, op0=OP.mult)
            nc.sync.dma_start(cc_in[:, m, HL:L], cc1[:])
        nc.gpsimd.collective_compute(
            "AllReduce", OP.add,
            replica_groups=[[0, 4], [1, 5], [2, 6], [3, 7]],
            ins=[cc_in.opt()], outs=[cc_out.opt()])

        bcasts(1, BCb, BCc)
        p567(1, BCb, BCc)
        fc2_w = w_big("fc2_wT", FF, D, q=nc.gpsimd)

        # =============== P9: combine halves + residual ======================
        pg = arena.tile([128, DT, L], BF16, name="pg", tag="s8b")
        nc.sync.dma_start(pg[:], cc_out[:])
        for m in range(DT):
            t0 = rot.tile([128, HL], BF16, name="sel_t0", tag="ada_xr", bufs=2)
            nc.vector.tensor_scalar(t0[:], p_bf[:, m, HL:L], msk[:, 0:1],
                                    None, op0=OP.mult)
            t1 = rot.tile([128, HL], BF16, name="sel_t1", tag="ada_xr", bufs=2)
            nc.vector.scalar_tensor_tensor(t1[:], _rev(p_bf[:, m, HL:L]),
                                           msk[:, 1:2], t0[:],
                                           op0=OP.mult, op1=OP.add)
            t2 = rot.tile([128, HL], BF16, name="sel_t2", tag="ada_xr", bufs=2)
            nc.vector.scalar_tensor_tensor(t2[:], pg[:, m, HL:L], msk[:, 0:1],
                                           t1[:], op0=OP.mult, op1=OP.add)
            t3 = rot.tile([128, HL], BF16, name="sel_t3", tag="ada_xr", bufs=2)
            nc.vector.scalar_tensor_tensor(t3[:], pg[:, m, 0:HL], msk[:, 1:2],
                                           t2[:], op0=OP.mult, op1=OP.add)
            nc.vector.scalar_tensor_tensor(x1[:, m, :], t3[:], ls1[:, m],
                                           x_half[:, m, :], op0=OP.mult,
                                           op1=OP.add)
        if debug:
            x1d = rot.tile([128, DT, HL], F32, name="x1d", tag="x1d", bufs=1)
            for m in range(DT):
                nc.vector.tensor_copy(x1d[:, m, :], x1[:, m, :])
            nc.sync.dma_start(
                dbg["x1_dbg"].rearrange("(a p) m -> p a m", p=128), x1d[:])

        # =============== P10: ada_norm 2 (precomputed gamma/beta) ===========
        ms2 = psum.tile([1, HL], F32, name="ms2", tag="msps", bufs=1)
        for i in range(DT):
            sq = rot.tile([128, HL], BF16, name="p10_sq", tag="ada_sq", bufs=2)
            nc.vector.tensor_tensor(sq[:], x1[:, i, :], x1[:, i, :], op=OP.mult)
            nc.tensor.matmul(ms2[:, :], ones_col[:], sq[:],
                             start=(i == 0), stop=(i == DT - 1))
        lnm2 = rot.tile([1, HL], BF16, name="lnm2", tag="ada_lnm", bufs=1)
        nc.scalar.activation(lnm2[:], ms2[:], AF.Ln, bias=eps_t[:], scale=1.0 / D)
        rinv2 = rot.tile([1, HL], BF16, name="rinv2", tag="ada_rinv", bufs=1)
        nc.scalar.activation(rinv2[:], lnm2[:], AF.Exp, bias=0.0, scale=-0.5)
        rb2 = rot.tile([128, HL], BF16, name="rb2", tag="ada_rb", bufs=1)
        nc.sync.dma_start(rb2[:], _bcast_row(rinv2[0:1, :]))
        h2 = arena.tile([128, DT, HL], BF16, name="h2", tag="sCall")
        for i in range(DT):
            xr = rot.tile([128, HL], BF16, name="p10_xr", tag="ada_xr", bufs=2)
            nc.vector.tensor_tensor(xr[:], x1[:, i, :], rb2[:, :], op=OP.mult)
            hp = rot.tile([128, HL], BF16, name="p10_hp", tag="ada_xr", bufs=2)
            nc.vector.tensor_tensor(hp[:], xr[:], s2gb[:, i, :], op=OP.mult)
            nc.vector.tensor_tensor(h2[:, i, :], hp[:], s2gb[:, DT + i, :],
                                    op=OP.add)

        # =============== P11: SwiGLU FFN ====================================
        sgf = arena.tile([128, FT, HL], BF16, name="sgf", tag="s16a")
        gv = arena.tile([128, FT, HL], BF16, name="gv", tag="s16c")
        for m in range(2 * FT):
            ps = psum.tile([128, HL], F32, name="p11ps", tag="y2ps")
            fw = fc1a if m < FT else fc1b
            mc = m if m < FT else m - FT
            for k in range(DT):
                nc.tensor.matmul(ps[:, :], fw[:, k, mc * 128:(mc + 1) * 128],
                                 h2[:, k, :], start=(k == 0), stop=(k == DT - 1))
            if m < FT:
                nc.scalar.activation(sgf[:, m, :], ps[:, :], AF.Silu,
                                     bias=fc1_b[:, m], scale=1.0)
            else:
                vv = rot.tile([128, HL], BF16, name="p11_vv", tag="ada_tg", bufs=2)
                nc.scalar.activation(vv[:], ps[:, :], AF.Identity,
                                     bias=fc1_b[:, m], scale=1.0)
                nc.vector.tensor_tensor(gv[:, m - FT, :], sgf[:, m - FT, :],
                                        vv[:], op=OP.mult)
        out_sb = arena.tile([128, DT, HL], F32, name="out_sb", tag="s8x")
        for m in range(DT):
            ps = psum.tile([128, HL], F32, name="p12ps", tag="y2ps")
            for k in range(FT):
                nc.tensor.matmul(ps[:, :], fc2_w[:, k, m * 128:(m + 1) * 128],
                                 gv[:, k, :], start=(k == 0), stop=False)
            nc.tensor.matmul(ps[:, :], c2row[:, m * 128:(m + 1) * 128],
                             ones_row[:], start=False, stop=True)
            nc.vector.scalar_tensor_tensor(out_sb[:, m, :], ps[:, :], ls2[:, m],
                                           x1[:, m, :], op0=OP.mult, op1=OP.add)
        nc.sync.dma_start(out_ap.rearrange("(a p) m -> p a m", p=128), out_sb[:])

    nc.compile()
    return nc, dbg


def _prep_inputs(inputs):
    """Host-side: per-core input dicts."""
    f32 = np.float32
    bf = ml_dtypes.bfloat16
    x = np.asarray(inputs["x"], f32)
    pn = np.asarray(inputs["phys_norm"], f32)
    blk_w = np.asarray(inputs["blk_w"], f32)
    ls1v = np.asarray(inputs["ls1"], f32)
    blk_b = np.asarray(inputs["blk_b"], f32)
    in_maps = []
    for c in range(NCORES):
        d, b = c // 4, c % 4
        xb = x[b] if d == 0 else x[b, ::-1]
        pnb = pn[b] if d == 0 else pn[b, ::-1]
        osl = slice(HL, L) if d == 0 else slice(0, HL)  # my output half (global)
        xh = x[b, osl] + (ls1v * blk_b)[None, :]
        Wd = blk_w[:, d * D:(d + 1) * D] @ np.asarray(inputs["m_out_w"][d], f32)
        m = {
            "xhT": np.ascontiguousarray(xh.T),
            "xbT": np.ascontiguousarray(xb.T).astype(bf),
            "pnT": np.ascontiguousarray(pnb.T).astype(bf),
            "pnoT": np.ascontiguousarray(pn[b, osl].T).astype(bf),
            "dirmask": np.tile(np.array([[1.0 - d, float(d)]], f32), (128, 1)),
            "eye": np.eye(128, dtype=f32).astype(bf),
            "in_wT": np.ascontiguousarray(inputs["m_in_w"][d].T).astype(bf),
            "conv_w": np.asarray(inputs["m_conv_w"][d], f32),
            "conv_b": np.asarray(inputs["m_conv_b"][d], f32).reshape(ED, 1),
            "xp_wT": np.ascontiguousarray(inputs["m_xproj_w"][d].T).astype(bf),
            "dt_wT": np.ascontiguousarray(inputs["m_dt_w"][d].T).astype(bf),
            "dt_b": np.asarray(inputs["m_dt_b"][d], f32).reshape(ED, 1),
            "A": (-np.exp(np.asarray(inputs["m_A_log"][d], f32))),
            "Dsk": np.asarray(inputs["m_D"][d], f32).reshape(ED, 1),
            "WdT": np.ascontiguousarray(Wd.T).astype(bf),
            "ls1": ls1v.reshape(D, 1),
            "fc1_wT": np.ascontiguousarray(inputs["fc1_w"].T).astype(bf),
            "fc1_b": np.asarray(inputs["fc1_b"], f32).reshape(2 * FF, 1),
            "fc2_wT": np.ascontiguousarray(inputs["fc2_w"].T).astype(bf),
            "ls2": np.asarray(inputs["ls2"], f32).reshape(D, 1),
            "c2T": np.asarray(inputs["fc2_b"], f32).reshape(1, D).astype(bf),
        }
        for p in ("n1", "n2"):
            m[p + "_w1T"] = np.ascontiguousarray(inputs[p + "_w1"].T).astype(bf)
            m[p + "_b1"] = np.asarray(inputs[p + "_b1"], f32).reshape(2 * D, 1)
            m[p + "_w2T"] = np.ascontiguousarray(inputs[p + "_w2"].T).astype(bf)
            m[p + "_b2"] = np.asarray(inputs[p + "_b2"], f32).reshape(2 * D, 1)
            m[p + "_sc"] = np.asarray(inputs[p + "_scale"], f32).reshape(D, 1)
        in_maps.append(m)
    return in_maps


def run(inputs, debug=False, trace=False):
    key = ("dbg" if debug else "lean")
    if key not in _cache:
        _cache[key] = build(debug=debug)
    nc, dbg = _cache[key]
    in_maps = _prep_inputs(inputs)
    res = run_bass_kernel_spmd(nc, in_maps, core_ids=list(range(NCORES)),
                               trace=trace)
    out = np.zeros((B, L, D), np.float32)
    for c in range(NCORES):
        d, b = c // 4, c % 4
        o = res.results[c]["out"]  # [D, HL], global order, my half
        if d == 0:
            out[b, HL:L] = o.T
        else:
            out[b, 0:HL] = o.T
    return out, res


def kernel(**inputs):
    out, _ = run(inputs, debug=False, trace=False)
    return out
